# revision 8
# baseline (speedup 1.0000x reference)
"""Trainium2 Bass kernel for nn_Attention_2216203124924 (sparse/varlen GQA attention).

Full computation:
  xq/xk/xv = x @ {wq,wk,wv}.T ; per-head RMSNorm(q,k) ; RoPE via
  rope_cache[positions] ; GQA repeat ; per-segment causal attention
  (segments from cu_seqlens) ; out @ wo.T

Distribution (8 NeuronCores, tensor-parallel over heads):
  core c owns q-heads [4c,4c+4) and kv-head c (GQA groups align),
  wo is row-sharded; each core emits a partial [2048,4096] output and the
  host sums the 8 partials.

On-device layout is "transposed" ([feature, seq]) throughout so the
contraction dim always sits on SBUF partitions:
  qT/kT/vT from weight-stationary projection matmuls, RMSNorm stats via
  ones-column matmul + matmul-broadcast of rsqrt row, RoPE as elementwise
  muls with host-gathered cos/sin (+ PE swap-half permutation), scores^T =
  kT_tile.T @ qT, probs via unnormalized exp (scores are O(1), max-sub
  unneeded) with compile-time segment mask plan, PV accumulated over key
  tiles in PSUM, normalization by matmul-broadcast reciprocal row, and the
  output projection from attnT tiles against wo^T.

All matmul operands are float32r (~13-bit mantissa, full PE rate).
The segment/causal structure from cu_seqlens and the rope gather by
positions are resolved on the host at build time; the NEFF is specialized
to them.
"""

import os
import sys

import numpy as np

for _p in ("/opt/trn_rl_repo",):
    if os.path.isdir(_p) and _p not in sys.path:
        sys.path.insert(0, _p)

S = 2048
D = 4096
HD = 128
HALF = 64
N_HEADS = 32
N_KV = 8
NCORES = 8
QH = N_HEADS // NCORES          # 4 q heads per core
NO = QH + 2                     # o-tiles per core in qkv projection: q0..q3, k, v
DT = D // 128                   # 32 contraction tiles
MC = S // 512                   # 4 m-chunks of 512
NT = S // 128                   # 16 key tiles
EPS = 1e-6
SCALE = HD ** -0.5

LAST_RESULT = None  # BassKernelResults of the most recent run (for test harness)


def _attention_plan(cu_seqlens):
    """Compile-time mask plan from cu_seqlens.

    Returns (plan, mask_pack):
      plan[mc] = list of (nt, w0, w1, mask_ops); w0/w1 are column offsets
      (multiples of 128, relative to the 512-wide m-chunk) of the contiguous
      valid window; mask_ops = [(j, kind, idx)] for 128-col subtiles needing
      a multiplicative 0/1 mask: kind 'tri' uses the shared causal triangle,
      kind 'host' uses mask_pack[:, idx*128:(idx+1)*128].
    """
    idx = np.arange(S)
    seg = np.searchsorted(np.asarray(cu_seqlens), idx, side="right") - 1
    mask_qk = (seg[:, None] == seg[None, :]) & (idx[:, None] >= idx[None, :])
    mask_t = mask_qk.T  # [n, m]

    plan = []
    tiles = []
    tile_ids = {}
    for mc in range(MC):
        entries = []
        for nt in range(NT):
            blk = mask_t[nt * 128:(nt + 1) * 128, mc * 512:(mc + 1) * 512]
            if not blk.any():
                continue
            js = [j for j in range(4) if blk[:, j * 128:(j + 1) * 128].any()]
            jlo, jhi = min(js), max(js)
            assert js == list(range(jlo, jhi + 1)), "valid window not contiguous"
            mops = []
            for j in range(jlo, jhi + 1):
                sub = blk[:, j * 128:(j + 1) * 128]
                if sub.all():
                    continue
                m0g = mc * 512 + j * 128
                n0g = nt * 128
                if m0g == n0g and np.array_equal(
                    sub, idx[:128][None, :] >= idx[:128][:, None]
                ):
                    mops.append((j, "tri", -1))
                else:
                    key = sub.tobytes()
                    if key not in tile_ids:
                        tile_ids[key] = len(tiles)
                        tiles.append(sub.astype(np.float32))
                    mops.append((j, "host", tile_ids[key]))
            entries.append((nt, jlo * 128, (jhi + 1) * 128, mops))
        assert entries, "every query row attends to at least itself"
        plan.append(entries)

    if tiles:
        mask_pack = np.concatenate(tiles, axis=1)
    else:
        mask_pack = np.zeros((128, 128), dtype=np.float32)
    return plan, np.ascontiguousarray(mask_pack)


def _build_graph(plan, n_mask_cols):
    import concourse.bass as bass  # noqa: PLC0415
    import concourse.mybir as mybir  # noqa: PLC0415
    import concourse.tile as tile  # noqa: PLC0415
    from concourse import bacc  # noqa: PLC0415
    from contextlib import ExitStack  # noqa: PLC0415

    f32 = mybir.dt.float32
    f32r = mybir.dt.float32r
    AF = mybir.ActivationFunctionType

    nc = bacc.Bacc()
    xT_p = nc.declare_dram_parameter("xT", [D, S], f32r, isOutput=False)
    wqkv_p = nc.declare_dram_parameter("w_qkv", [128, NO * DT * 128], f32r, isOutput=False)
    wo_p = nc.declare_dram_parameter("w_o", [128, QH * D], f32r, isOutput=False)
    cs_p = nc.declare_dram_parameter("cs", [128, 4 * S], f32r, isOutput=False)
    mask_p = nc.declare_dram_parameter("mask_pack", [128, n_mask_cols], f32r, isOutput=False)
    consts_p = nc.declare_dram_parameter("consts", [128, 5 * 128], f32r, isOutput=False)
    out_p = nc.declare_dram_parameter("out", [S, D], f32, isOutput=True)

    with tile.TileContext(nc) as tc, ExitStack() as ctx:
        const = ctx.enter_context(tc.tile_pool(name="const", bufs=1))
        persist = ctx.enter_context(tc.tile_pool(name="persist", bufs=1))

        consts = const.tile([128, 5 * 128], f32r)
        nc.sync.dma_start(consts[:], consts_p[:])
        ones_col = consts[:, 0:1]
        ones_row = consts[0:1, 0:128]
        swp = consts[:, 128:256]        # swap-halves permutation
        ident = consts[:, 256:384]      # identity (for PE transpose)
        tri = consts[:, 384:512]        # causal triangle in [n, m]: 1 iff m >= n
        sca_row = consts[0:1, 512:640]  # all = HD**-0.5

        mask_sb = const.tile([128, n_mask_cols], f32r)
        nc.sync.dma_start(mask_sb[:], mask_p[:])

        eps_col = const.tile([128, 1], f32)
        nc.gpsimd.memset(eps_col[:], EPS)

        # persistent activations: q0..q3, k, v in transposed [feat, seq] layout
        qkvT = [persist.tile([128, S], f32r, tag=f"qkvT{o}", name=f"qkvT{o}") for o in range(NO)]

        # ---------------- stage 1: qkv projection + rms stats ----------------
        with ExitStack() as s1:
            pw = s1.enter_context(tc.tile_pool(name="wqkv", bufs=1))
            px = s1.enter_context(tc.tile_pool(name="xstream", bufs=3))
            pcs = s1.enter_context(tc.tile_pool(name="csstream", bufs=2))
            psc = s1.enter_context(tc.tile_pool(name="s1scratch", bufs=2))
            pq = s1.enter_context(tc.tile_pool(name="qkvpsum", bufs=1, space="PSUM"))
            pss = s1.enter_context(tc.tile_pool(name="ssqpsum", bufs=1, space="PSUM"))

            w_sb = pw.tile([128, NO * DT * 128], f32r)
            nc.sync.dma_start(w_sb[:], wqkv_p[:])

            for mc in range(MC):
                msl = slice(mc * 512, (mc + 1) * 512)
                accs = [pq.tile([128, 512], f32, tag=f"acc{o}", name=f"acc{o}") for o in range(NO)]
                for d in range(DT):
                    xt = px.tile([128, 512], f32r, tag="xt")
                    nc.sync.dma_start(xt[:], xT_p[d * 128:(d + 1) * 128, msl])
                    for o in range(NO):
                        woff = (o * DT + d) * 128
                        nc.tensor.matmul(
                            accs[o][:],
                            w_sb[:, woff:woff + 128],
                            xt[:],
                            start=(d == 0),
                            stop=(d == DT - 1),
                        )
                for o in range(NO):
                    nc.vector.tensor_copy(qkvT[o][:, msl], accs[o][:])
                # rms stats + rope + scaling for q heads and k (v passes through)
                for o in range(QH + 1):
                    sq = psc.tile([128, 512], f32r, tag="sq")
                    nc.scalar.activation(sq[:], accs[o][:], AF.Square)
                    ss = pss.tile([1, 512], f32, tag="ss")
                    nc.tensor.matmul(ss[:], ones_col, sq[:], start=True, stop=True)
                    rsq = psc.tile([1, 512], f32, tag="rsq")
                    nc.scalar.activation(
                        rsq[:], ss[:], AF.Sqrt, bias=eps_col[0:1, :], scale=1.0 / HD
                    )
                    rsr = psc.tile([1, 512], f32r, tag="rsr")
                    with nc.allow_low_precision(reason="f32r rounding of rsqrt"):
                        nc.vector.reciprocal(rsr[:], rsq[:])

                    csb = 0 if o < QH else 2  # q heads share cs1q/cs2q; k: cs1k/cs2k
                    row = sca_row if o < QH else ones_row  # fold attn scale into q
                    cs1 = pcs.tile([128, 512], f32r, tag="cs1")
                    cs2 = pcs.tile([128, 512], f32r, tag="cs2")
                    nc.sync.dma_start(cs1[:], cs_p[:, csb * S + mc * 512: csb * S + (mc + 1) * 512])
                    nc.sync.dma_start(cs2[:], cs_p[:, (csb + 1) * S + mc * 512: (csb + 1) * S + (mc + 1) * 512])
                    # B = swap_halves(qT) via PE permutation
                    bp = pq.tile([128, 512], f32, tag=f"acc{o}", name=f"bp{o}")
                    nc.tensor.matmul(bp[:], swp, qkvT[o][:, msl], start=True, stop=True)
                    t1 = psc.tile([128, 512], f32, tag="t1")
                    nc.vector.tensor_mul(t1[:], qkvT[o][:, msl], cs1[:])
                    t2 = psc.tile([128, 512], f32, tag="t2")
                    nc.vector.tensor_mul(t2[:], bp[:], cs2[:])
                    nc.vector.tensor_add(t1[:], t1[:], t2[:])
                    # broadcast rs row across partitions
                    bc = pss.tile([128, 512], f32, tag="bc", name="bc")
                    nc.tensor.matmul(bc[:], row, rsr[:], start=True, stop=True)
                    nc.vector.tensor_mul(qkvT[o][:, msl], t1[:], bc[:])

        # ---------------- stage 2: attention ----------------
        with ExitStack() as s2:
            p2 = s2.enter_context(tc.tile_pool(name="persist2", bufs=1))
            v_sb = p2.tile([128, S], f32r)
            attnT = [p2.tile([128, S], f32r, tag=f"attnT{h}", name=f"attnT{h}") for h in range(QH)]
            wo_sb = p2.tile([128, QH * D], f32r)
            nc.sync.dma_start(wo_sb[:], wo_p[:])

            kT = qkvT[QH]
            vT = qkvT[QH + 1]

            with ExitStack() as s2a:
                ptp = s2a.enter_context(tc.tile_pool(name="tppsum", bufs=2, space="PSUM"))
                for nt in range(NT):
                    nsl = slice(nt * 128, (nt + 1) * 128)
                    tp = ptp.tile([128, 128], f32, tag="tp")
                    nc.tensor.transpose(
                        tp[:], vT[:, nsl].bitcast(mybir.dt.float32), ident.bitcast(mybir.dt.float32)
                    )
                    nc.vector.tensor_copy(v_sb[:, nsl], tp[:])

            with ExitStack() as s2b:
                psco = s2b.enter_context(tc.tile_pool(name="scpsum", bufs=2, space="PSUM"))
                pov = s2b.enter_context(tc.tile_pool(name="ovpsum", bufs=2, space="PSUM"))
                pden = s2b.enter_context(tc.tile_pool(name="denpsum", bufs=2, space="PSUM"))
                pbc2 = s2b.enter_context(tc.tile_pool(name="bc2psum", bufs=1, space="PSUM"))
                pex = s2b.enter_context(tc.tile_pool(name="exsbuf", bufs=3))
                pnr = s2b.enter_context(tc.tile_pool(name="nrsbuf", bufs=2))

                for h in range(QH):
                    for mc in range(MC):
                        entries = plan[mc]
                        ov = pov.tile([128, 512], f32, tag="ov")
                        den = pden.tile([1, 512], f32, tag="den")
                        n_ent = len(entries)
                        for i, (nt, w0, w1, mops) in enumerate(entries):
                            nsl = slice(nt * 128, (nt + 1) * 128)
                            qsl = slice(mc * 512 + w0, mc * 512 + w1)
                            sc = psco.tile([128, 512], f32, tag="sc")
                            nc.tensor.matmul(
                                sc[:, w0:w1], kT[:, nsl], qkvT[h][:, qsl],
                                start=True, stop=True,
                            )
                            ex = pex.tile([128, 512], f32r, tag="ex")
                            nc.scalar.activation(ex[:, w0:w1], sc[:, w0:w1], AF.Exp)
                            for (j, kind, tix) in mops:
                                jsl = slice(j * 128, (j + 1) * 128)
                                msrc = tri if kind == "tri" else mask_sb[:, tix * 128:(tix + 1) * 128]
                                nc.vector.tensor_mul(ex[:, jsl], ex[:, jsl], msrc)
                            first = i == 0
                            last = i == n_ent - 1
                            nc.tensor.matmul(
                                ov[:, w0:w1], v_sb[:, nsl], ex[:, w0:w1],
                                start=first, stop=last, skip_group_check=True,
                            )
                            nc.tensor.matmul(
                                den[0:1, w0:w1], ones_col, ex[:, w0:w1],
                                start=first, stop=last, skip_group_check=True,
                            )
                        rd = pnr.tile([1, 512], f32r, tag="rd")
                        with nc.allow_low_precision(reason="f32r rounding of softmax denom"):
                            nc.vector.reciprocal(rd[:], den[:])
                        bc = pbc2.tile([128, 512], f32, tag="bc2")
                        nc.tensor.matmul(bc[:], ones_row, rd[:], start=True, stop=True)
                        bcs = pnr.tile([128, 512], f32, tag="bcs")
                        nc.vector.tensor_copy(bcs[:], bc[:])
                        nc.vector.tensor_mul(
                            attnT[h][:, mc * 512:(mc + 1) * 512], ov[:], bcs[:]
                        )

            # ---------------- stage 3: output projection ----------------
            with ExitStack() as s3:
                py = s3.enter_context(tc.tile_pool(name="ypsum", bufs=4, space="PSUM"))
                pys = s3.enter_context(tc.tile_pool(name="ysbuf", bufs=3))
                for mt in range(S // 128):
                    tsl = slice(mt * 128, (mt + 1) * 128)
                    for ec in range(D // 512):
                        yp = py.tile([128, 512], f32, tag="yp")
                        for t in range(QH):
                            nc.tensor.matmul(
                                yp[:],
                                attnT[t][:, tsl],
                                wo_sb[:, t * D + ec * 512: t * D + (ec + 1) * 512],
                                start=(t == 0),
                                stop=(t == QH - 1),
                            )
                        ys = pys.tile([128, 512], f32, tag="ys")
                        nc.vector.tensor_copy(ys[:], yp[:])
                        nc.sync.dma_start(out_p[tsl, ec * 512:(ec + 1) * 512], ys[:])

    nc.finalize()
    return nc


def kernel(x, wq, wk, wv, wo, q_norm_w, k_norm_w, rope_cache, positions, cu_seqlens):
    global LAST_RESULT
    from concourse.bass_utils import run_bass_kernel_spmd  # noqa: PLC0415

    x = np.asarray(x, dtype=np.float32)
    wq = np.asarray(wq, dtype=np.float32)
    wk = np.asarray(wk, dtype=np.float32)
    wv = np.asarray(wv, dtype=np.float32)
    wo = np.asarray(wo, dtype=np.float32)
    q_norm_w = np.asarray(q_norm_w, dtype=np.float32)
    k_norm_w = np.asarray(k_norm_w, dtype=np.float32)
    rope_cache = np.asarray(rope_cache, dtype=np.float32)
    positions = np.asarray(positions)
    cu_seqlens = np.asarray(cu_seqlens)

    # ---- host prep (shared) ----
    xT = np.ascontiguousarray(x[0].T)  # [D, S]

    pos = positions.reshape(-1)
    cs = rope_cache[pos]               # [S, HALF, 2]
    cosT = cs[:, :, 0].T               # [HALF, S]
    sinT = cs[:, :, 1].T
    cs1 = np.concatenate([cosT, cosT], axis=0)    # [128, S]
    cs2 = np.concatenate([-sinT, sinT], axis=0)

    def fold(w):
        w = w.reshape(HD, 1)
        wsw = np.concatenate([w[HALF:], w[:HALF]], axis=0)
        return cs1 * w, cs2 * wsw

    cs1q, cs2q = fold(q_norm_w)
    cs1k, cs2k = fold(k_norm_w)
    cs_host = np.ascontiguousarray(
        np.concatenate([cs1q, cs2q, cs1k, cs2k], axis=1), dtype=np.float32
    )  # [128, 4S]

    plan, mask_pack = _attention_plan(cu_seqlens)

    consts = np.zeros((128, 5 * 128), dtype=np.float32)
    consts[:, 0:128] = 1.0
    swp = np.zeros((128, 128), dtype=np.float32)
    swp[np.arange(128), (np.arange(128) + HALF) % 128] = 1.0
    consts[:, 128:256] = swp
    consts[:, 256:384] = np.eye(128, dtype=np.float32)
    consts[:, 384:512] = np.triu(np.ones((128, 128), dtype=np.float32))
    consts[:, 512:640] = SCALE

    # ---- per-core weight shards ----
    in_maps = []
    for c in range(NCORES):
        w_all = np.concatenate(
            [
                wq[c * QH * HD:(c + 1) * QH * HD],   # [512, D]
                wk[c * HD:(c + 1) * HD],             # [128, D]
                wv[c * HD:(c + 1) * HD],             # [128, D]
            ],
            axis=0,
        )  # [NO*128, D]
        w_host = np.ascontiguousarray(
            w_all.reshape(NO, 128, DT, 128).transpose(3, 0, 2, 1).reshape(128, NO * DT * 128)
        )
        wo_c = wo[:, c * QH * HD:(c + 1) * QH * HD].T  # [512, D]
        wo_host = np.ascontiguousarray(
            wo_c.reshape(QH, 128, D).transpose(1, 0, 2).reshape(128, QH * D)
        )
        in_maps.append(
            {
                "xT": xT,
                "w_qkv": w_host,
                "w_o": wo_host,
                "cs": cs_host,
                "mask_pack": mask_pack,
                "consts": consts,
            }
        )

    nc = _build_graph(plan, mask_pack.shape[1])
    res = run_bass_kernel_spmd(nc, in_maps, list(range(NCORES)))
    LAST_RESULT = res

    out = res.results[0]["out"].astype(np.float32)
    for c in range(1, NCORES):
        out = out + res.results[c]["out"]
    return out.reshape(1, S, D)


# revision 10
# speedup vs baseline: 1.1102x; 1.1102x over previous
"""Trainium2 Bass kernel for nn_Attention_2216203124924 (sparse/varlen GQA attention).

Full computation:
  xq/xk/xv = x @ {wq,wk,wv}.T ; per-head RMSNorm(q,k) ; RoPE via
  rope_cache[positions] ; GQA repeat ; per-segment causal attention
  (segments from cu_seqlens) ; out @ wo.T

Distribution (8 NeuronCores, tensor-parallel over heads):
  core c owns q-heads [4c,4c+4) and kv-head c (GQA groups align),
  wo is row-sharded; each core emits a partial [2048,4096] output and the
  host sums the 8 partials.

On-device layout is "transposed" ([feature, seq]) throughout so the
contraction dim always sits on SBUF partitions:
  qT/kT/vT from weight-stationary projection matmuls, RMSNorm stats via
  ones-column matmul + matmul-broadcast of rsqrt row, RoPE as elementwise
  muls with host-gathered cos/sin (+ PE swap-half permutation), scores^T =
  kT_tile.T @ qT, probs via unnormalized exp (scores are O(1), max-sub
  unneeded) with compile-time segment mask plan, PV accumulated over key
  tiles in PSUM, normalization by matmul-broadcast reciprocal row, and the
  output projection from attnT tiles against wo^T.

All matmul operands are float32r (~13-bit mantissa, full PE rate).
The segment/causal structure from cu_seqlens and the rope gather by
positions are resolved on the host at build time; the NEFF is specialized
to them.
"""

import os
import sys

import numpy as np

for _p in ("/opt/trn_rl_repo",):
    if os.path.isdir(_p) and _p not in sys.path:
        sys.path.insert(0, _p)

S = 2048
D = 4096
HD = 128
HALF = 64
N_HEADS = 32
N_KV = 8
NCORES = 8
QH = N_HEADS // NCORES          # 4 q heads per core
NO = QH + 2                     # o-tiles per core in qkv projection: q0..q3, k, v
DT = D // 128                   # 32 contraction tiles
MC = S // 512                   # 4 m-chunks of 512
NT = S // 128                   # 16 key tiles
EPS = 1e-6
SCALE = HD ** -0.5

LAST_RESULT = None  # BassKernelResults of the most recent run (for test harness)


def _attention_plan(cu_seqlens):
    """Compile-time mask plan from cu_seqlens.

    Returns (plan, mask_pack):
      plan[mc] = list of (nt, w0, w1, mask_ops); w0/w1 are column offsets
      (multiples of 128, relative to the 512-wide m-chunk) of the contiguous
      valid window; mask_ops = [(j, kind, idx)] for 128-col subtiles needing
      a multiplicative 0/1 mask: kind 'tri' uses the shared causal triangle,
      kind 'host' uses mask_pack[:, idx*128:(idx+1)*128].
    """
    idx = np.arange(S)
    seg = np.searchsorted(np.asarray(cu_seqlens), idx, side="right") - 1
    mask_qk = (seg[:, None] == seg[None, :]) & (idx[:, None] >= idx[None, :])
    mask_t = mask_qk.T  # [n, m]

    plan = []
    tiles = []
    tile_ids = {}
    for mc in range(MC):
        entries = []
        for nt in range(NT):
            blk = mask_t[nt * 128:(nt + 1) * 128, mc * 512:(mc + 1) * 512]
            if not blk.any():
                continue
            js = [j for j in range(4) if blk[:, j * 128:(j + 1) * 128].any()]
            jlo, jhi = min(js), max(js)
            assert js == list(range(jlo, jhi + 1)), "valid window not contiguous"
            mops = []
            for j in range(jlo, jhi + 1):
                sub = blk[:, j * 128:(j + 1) * 128]
                if sub.all():
                    continue
                m0g = mc * 512 + j * 128
                n0g = nt * 128
                if m0g == n0g and np.array_equal(
                    sub, idx[:128][None, :] >= idx[:128][:, None]
                ):
                    mops.append((j, "tri", -1))
                else:
                    key = sub.tobytes()
                    if key not in tile_ids:
                        tile_ids[key] = len(tiles)
                        tiles.append(sub.astype(np.float32))
                    mops.append((j, "host", tile_ids[key]))
            entries.append((nt, jlo * 128, (jhi + 1) * 128, mops))
        assert entries, "every query row attends to at least itself"
        plan.append(entries)

    if tiles:
        mask_pack = np.concatenate(tiles, axis=1)
    else:
        mask_pack = np.zeros((128, 128), dtype=np.float32)
    return plan, np.ascontiguousarray(mask_pack)


def _build_graph(plan, n_mask_cols):
    import concourse.bass as bass  # noqa: PLC0415
    import concourse.mybir as mybir  # noqa: PLC0415
    import concourse.tile as tile  # noqa: PLC0415
    from concourse import bacc  # noqa: PLC0415
    from contextlib import ExitStack  # noqa: PLC0415

    f32 = mybir.dt.float32
    f32r = mybir.dt.float32r
    AF = mybir.ActivationFunctionType

    nc = bacc.Bacc()
    xT_p = nc.declare_dram_parameter("xT", [D, S], f32r, isOutput=False)
    wqkv_p = nc.declare_dram_parameter("w_qkv", [128, NO * DT * 128], f32r, isOutput=False)
    wo_p = nc.declare_dram_parameter("w_o", [128, QH * D], f32r, isOutput=False)
    cs_p = nc.declare_dram_parameter("cs", [128, 4 * S], f32r, isOutput=False)
    mask_p = nc.declare_dram_parameter("mask_pack", [128, n_mask_cols], f32r, isOutput=False)
    consts_p = nc.declare_dram_parameter("consts", [128, 5 * 128], f32r, isOutput=False)
    out_p = nc.declare_dram_parameter("out", [S, D], f32, isOutput=True)

    with tile.TileContext(nc) as tc, ExitStack() as ctx:
        const = ctx.enter_context(tc.tile_pool(name="const", bufs=1))
        persist = ctx.enter_context(tc.tile_pool(name="persist", bufs=1))

        consts = const.tile([128, 5 * 128], f32r)
        nc.sync.dma_start(consts[:], consts_p[:])
        ones_col = consts[:, 0:1]
        ones_row = consts[0:1, 0:128]
        swp = consts[:, 128:256]        # swap-halves permutation
        ident = consts[:, 256:384]      # identity (for PE transpose)
        tri = consts[:, 384:512]        # causal triangle in [n, m]: 1 iff m >= n
        sca_row = consts[0:1, 512:640]  # all = HD**0.5 (divide-by folds the attn scale)

        mask_sb = const.tile([128, n_mask_cols], f32r)
        nc.sync.dma_start(mask_sb[:], mask_p[:])

        eps_col = const.tile([128, 1], f32)
        nc.gpsimd.memset(eps_col[:], EPS)

        # persistent activations: q0..q3, k, v in transposed [feat, seq] layout
        qkvT = [persist.tile([128, S], f32r, tag=f"qkvT{o}", name=f"qkvT{o}") for o in range(NO)]

        # ---------------- stage 1: qkv projection + rms stats ----------------
        with ExitStack() as s1:
            pw = s1.enter_context(tc.tile_pool(name="wqkv", bufs=1))
            px = s1.enter_context(tc.tile_pool(name="xstream", bufs=3))
            pcs = s1.enter_context(tc.tile_pool(name="csstream", bufs=2))
            psc = s1.enter_context(tc.tile_pool(name="s1scratch", bufs=2))
            pq = s1.enter_context(tc.tile_pool(name="qkvpsum", bufs=1, space="PSUM"))
            pss = s1.enter_context(tc.tile_pool(name="ssqpsum", bufs=1, space="PSUM"))

            w_sb = pw.tile([128, NO * DT * 128], f32r)
            nc.sync.dma_start(w_sb[:], wqkv_p[:])

            for mc in range(MC):
                msl = slice(mc * 512, (mc + 1) * 512)
                accs = [pq.tile([128, 512], f32, tag=f"acc{o}", name=f"acc{o}") for o in range(NO)]
                for d in range(DT):
                    xt = px.tile([128, 512], f32r, tag="xt")
                    nc.sync.dma_start(xt[:], xT_p[d * 128:(d + 1) * 128, msl])
                    for o in range(NO):
                        woff = (o * DT + d) * 128
                        nc.tensor.matmul(
                            accs[o][:],
                            w_sb[:, woff:woff + 128],
                            xt[:],
                            start=(d == 0),
                            stop=(d == DT - 1),
                        )
                for o in range(NO):
                    nc.scalar.activation(qkvT[o][:, msl], accs[o][:], AF.Copy)
                # rms stats + rope + scaling for q heads and k (v passes through)
                for o in range(QH + 1):
                    sq = psc.tile([128, 512], f32r, tag="sq")
                    nc.scalar.activation(sq[:], accs[o][:], AF.Square)
                    ss = pss.tile([1, 512], f32, tag="ss")
                    nc.tensor.matmul(ss[:], ones_col, sq[:], start=True, stop=True)
                    rsq = psc.tile([1, 512], f32r, tag="rsq")
                    nc.scalar.activation(
                        rsq[:], ss[:], AF.Sqrt, bias=eps_col[0:1, :], scale=1.0 / HD
                    )

                    csb = 0 if o < QH else 2  # q heads share cs1q/cs2q; k: cs1k/cs2k
                    row = sca_row if o < QH else ones_row  # fold attn scale into q
                    cs1 = pcs.tile([128, 512], f32r, tag="cs1")
                    cs2 = pcs.tile([128, 512], f32r, tag="cs2")
                    nc.sync.dma_start(cs1[:], cs_p[:, csb * S + mc * 512: csb * S + (mc + 1) * 512])
                    nc.sync.dma_start(cs2[:], cs_p[:, (csb + 1) * S + mc * 512: (csb + 1) * S + (mc + 1) * 512])
                    # B = swap_halves(qT) via PE permutation
                    bp = pq.tile([128, 512], f32, tag=f"acc{o}", name=f"bp{o}")
                    nc.tensor.matmul(bp[:], swp, qkvT[o][:, msl], start=True, stop=True)
                    t1 = psc.tile([128, 512], f32, tag="t1")
                    nc.vector.tensor_mul(t1[:], qkvT[o][:, msl], cs1[:])
                    t2 = psc.tile([128, 512], f32, tag="t2")
                    nc.vector.tensor_mul(t2[:], bp[:], cs2[:])
                    nc.vector.tensor_add(t1[:], t1[:], t2[:])
                    # broadcast (1/scale)*sqrt(var) across partitions, invert, multiply
                    bc = pss.tile([128, 512], f32, tag="bc", name="bc")
                    nc.tensor.matmul(bc[:], row, rsq[:], start=True, stop=True)
                    rrb = psc.tile([128, 512], f32, tag="rrb")
                    nc.vector.reciprocal_approx_fast(out=rrb[:], in_=bc[:])
                    nc.vector.tensor_mul(qkvT[o][:, msl], t1[:], rrb[:])

        # ---------------- stage 2: attention ----------------
        with ExitStack() as s2:
            p2 = s2.enter_context(tc.tile_pool(name="persist2", bufs=1))
            v_sb = p2.tile([128, S], f32r)
            attnT = [p2.tile([128, S], f32r, tag=f"attnT{h}", name=f"attnT{h}") for h in range(QH)]
            wo_sb = p2.tile([128, QH * D], f32r)
            nc.sync.dma_start(wo_sb[:], wo_p[:])

            kT = qkvT[QH]
            vT = qkvT[QH + 1]

            with ExitStack() as s2a:
                ptp = s2a.enter_context(tc.tile_pool(name="tppsum", bufs=2, space="PSUM"))
                for nt in range(NT):
                    nsl = slice(nt * 128, (nt + 1) * 128)
                    tp = ptp.tile([128, 128], f32, tag="tp")
                    nc.tensor.transpose(
                        tp[:], vT[:, nsl].bitcast(mybir.dt.float32), ident.bitcast(mybir.dt.float32)
                    )
                    nc.scalar.activation(v_sb[:, nsl], tp[:], AF.Copy)

            with ExitStack() as s2b:
                psco = s2b.enter_context(tc.tile_pool(name="scpsum", bufs=3, space="PSUM"))
                pov = s2b.enter_context(tc.tile_pool(name="ovpsum", bufs=2, space="PSUM"))
                pden = s2b.enter_context(tc.tile_pool(name="denpsum", bufs=2, space="PSUM"))
                pbc2 = s2b.enter_context(tc.tile_pool(name="bc2psum", bufs=1, space="PSUM"))
                pex = s2b.enter_context(tc.tile_pool(name="exsbuf", bufs=3))
                pnr = s2b.enter_context(tc.tile_pool(name="nrsbuf", bufs=2))

                for h in range(QH):
                    for mc in range(MC):
                        entries = plan[mc]
                        ov = pov.tile([128, 512], f32, tag="ov")
                        den = pden.tile([1, 512], f32, tag="den")
                        n_ent = len(entries)
                        for i, (nt, w0, w1, mops) in enumerate(entries):
                            nsl = slice(nt * 128, (nt + 1) * 128)
                            qsl = slice(mc * 512 + w0, mc * 512 + w1)
                            sc = psco.tile([128, 512], f32, tag="sc")
                            nc.tensor.matmul(
                                sc[:, w0:w1], kT[:, nsl], qkvT[h][:, qsl],
                                start=True, stop=True,
                            )
                            ex = pex.tile([128, 512], f32r, tag="ex")
                            nc.scalar.activation(ex[:, w0:w1], sc[:, w0:w1], AF.Exp)
                            for (j, kind, tix) in mops:
                                jsl = slice(j * 128, (j + 1) * 128)
                                msrc = tri if kind == "tri" else mask_sb[:, tix * 128:(tix + 1) * 128]
                                nc.vector.tensor_mul(ex[:, jsl], ex[:, jsl], msrc)
                            first = i == 0
                            last = i == n_ent - 1
                            nc.tensor.matmul(
                                ov[:, w0:w1], v_sb[:, nsl], ex[:, w0:w1],
                                start=first, stop=last, skip_group_check=True,
                            )
                            nc.tensor.matmul(
                                den[0:1, w0:w1], ones_col, ex[:, w0:w1],
                                start=first, stop=last, skip_group_check=True,
                            )
                        den_sb = pnr.tile([1, 512], f32r, tag="den_sb")
                        nc.scalar.activation(den_sb[:], den[:], AF.Copy)
                        bc = pbc2.tile([128, 512], f32, tag="bc2")
                        nc.tensor.matmul(bc[:], ones_row, den_sb[:], start=True, stop=True)
                        bcs = pnr.tile([128, 512], f32, tag="bcs")
                        nc.vector.reciprocal_approx_fast(out=bcs[:], in_=bc[:])
                        nc.vector.tensor_mul(
                            attnT[h][:, mc * 512:(mc + 1) * 512], ov[:], bcs[:]
                        )

            # ---------------- stage 3: output projection ----------------
            with ExitStack() as s3:
                py = s3.enter_context(tc.tile_pool(name="ypsum", bufs=4, space="PSUM"))
                pys = s3.enter_context(tc.tile_pool(name="ysbuf", bufs=3))
                for mt in range(S // 128):
                    tsl = slice(mt * 128, (mt + 1) * 128)
                    for ec in range(D // 512):
                        yp = py.tile([128, 512], f32, tag="yp")
                        for t in range(QH):
                            nc.tensor.matmul(
                                yp[:],
                                attnT[t][:, tsl],
                                wo_sb[:, t * D + ec * 512: t * D + (ec + 1) * 512],
                                start=(t == 0),
                                stop=(t == QH - 1),
                            )
                        ys = pys.tile([128, 512], f32, tag="ys")
                        nc.scalar.activation(ys[:], yp[:], AF.Copy)
                        nc.sync.dma_start(out_p[tsl, ec * 512:(ec + 1) * 512], ys[:])

    nc.finalize()
    return nc


def kernel(x, wq, wk, wv, wo, q_norm_w, k_norm_w, rope_cache, positions, cu_seqlens):
    global LAST_RESULT
    from concourse.bass_utils import run_bass_kernel_spmd  # noqa: PLC0415

    x = np.asarray(x, dtype=np.float32)
    wq = np.asarray(wq, dtype=np.float32)
    wk = np.asarray(wk, dtype=np.float32)
    wv = np.asarray(wv, dtype=np.float32)
    wo = np.asarray(wo, dtype=np.float32)
    q_norm_w = np.asarray(q_norm_w, dtype=np.float32)
    k_norm_w = np.asarray(k_norm_w, dtype=np.float32)
    rope_cache = np.asarray(rope_cache, dtype=np.float32)
    positions = np.asarray(positions)
    cu_seqlens = np.asarray(cu_seqlens)

    # ---- host prep (shared) ----
    xT = np.ascontiguousarray(x[0].T)  # [D, S]

    pos = positions.reshape(-1)
    cs = rope_cache[pos]               # [S, HALF, 2]
    cosT = cs[:, :, 0].T               # [HALF, S]
    sinT = cs[:, :, 1].T
    cs1 = np.concatenate([cosT, cosT], axis=0)    # [128, S]
    cs2 = np.concatenate([-sinT, sinT], axis=0)

    def fold(w):
        w = w.reshape(HD, 1)
        wsw = np.concatenate([w[HALF:], w[:HALF]], axis=0)
        return cs1 * w, cs2 * wsw

    cs1q, cs2q = fold(q_norm_w)
    cs1k, cs2k = fold(k_norm_w)
    cs_host = np.ascontiguousarray(
        np.concatenate([cs1q, cs2q, cs1k, cs2k], axis=1), dtype=np.float32
    )  # [128, 4S]

    plan, mask_pack = _attention_plan(cu_seqlens)

    consts = np.zeros((128, 5 * 128), dtype=np.float32)
    consts[:, 0:128] = 1.0
    swp = np.zeros((128, 128), dtype=np.float32)
    swp[np.arange(128), (np.arange(128) + HALF) % 128] = 1.0
    consts[:, 128:256] = swp
    consts[:, 256:384] = np.eye(128, dtype=np.float32)
    consts[:, 384:512] = np.triu(np.ones((128, 128), dtype=np.float32))
    consts[:, 512:640] = 1.0 / SCALE

    # ---- per-core weight shards ----
    in_maps = []
    for c in range(NCORES):
        w_all = np.concatenate(
            [
                wq[c * QH * HD:(c + 1) * QH * HD],   # [512, D]
                wk[c * HD:(c + 1) * HD],             # [128, D]
                wv[c * HD:(c + 1) * HD],             # [128, D]
            ],
            axis=0,
        )  # [NO*128, D]
        w_host = np.ascontiguousarray(
            w_all.reshape(NO, 128, DT, 128).transpose(3, 0, 2, 1).reshape(128, NO * DT * 128)
        )
        wo_c = wo[:, c * QH * HD:(c + 1) * QH * HD].T  # [512, D]
        wo_host = np.ascontiguousarray(
            wo_c.reshape(QH, 128, D).transpose(1, 0, 2).reshape(128, QH * D)
        )
        in_maps.append(
            {
                "xT": xT,
                "w_qkv": w_host,
                "w_o": wo_host,
                "cs": cs_host,
                "mask_pack": mask_pack,
                "consts": consts,
            }
        )

    nc = _build_graph(plan, mask_pack.shape[1])
    res = run_bass_kernel_spmd(nc, in_maps, list(range(NCORES)))
    LAST_RESULT = res

    out = res.results[0]["out"].astype(np.float32)
    for c in range(1, NCORES):
        out = out + res.results[c]["out"]
    return out.reshape(1, S, D)


# revision 11
# speedup vs baseline: 1.2953x; 1.1666x over previous
"""Trainium2 Bass kernel for nn_Attention_2216203124924 (sparse/varlen GQA attention).

Full computation:
  xq/xk/xv = x @ {wq,wk,wv}.T ; per-head RMSNorm(q,k) ; RoPE via
  rope_cache[positions] ; GQA repeat ; per-segment causal attention
  (segments from cu_seqlens) ; out @ wo.T

Distribution (8 NeuronCores, tensor-parallel over heads):
  core c owns q-heads [4c,4c+4) and kv-head c (GQA groups align),
  wo is row-sharded; each core emits a partial [2048,4096] output and the
  host sums the 8 partials.

On-device layout is "transposed" ([feature, seq]) throughout so the
contraction dim always sits on SBUF partitions:
  qT/kT/vT from weight-stationary projection matmuls, RMSNorm stats via
  ones-column matmul + matmul-broadcast of rsqrt row, RoPE as elementwise
  muls with host-gathered cos/sin (+ PE swap-half permutation), scores^T =
  kT_tile.T @ qT, probs via unnormalized exp (scores are O(1), max-sub
  unneeded) with compile-time segment mask plan, PV accumulated over key
  tiles in PSUM, normalization by matmul-broadcast reciprocal row, and the
  output projection from attnT tiles against wo^T.

All matmul operands are float32r (~13-bit mantissa, full PE rate).
The segment/causal structure from cu_seqlens and the rope gather by
positions are resolved on the host at build time; the NEFF is specialized
to them.
"""

import os
import sys

import numpy as np

for _p in ("/opt/trn_rl_repo",):
    if os.path.isdir(_p) and _p not in sys.path:
        sys.path.insert(0, _p)

S = 2048
D = 4096
HD = 128
HALF = 64
N_HEADS = 32
N_KV = 8
NCORES = 8
QH = N_HEADS // NCORES          # 4 q heads per core
NO = QH + 2                     # o-tiles per core in qkv projection: q0..q3, k, v
DT = D // 128                   # 32 contraction tiles
MC = S // 512                   # 4 m-chunks of 512
NT = S // 128                   # 16 key tiles
EPS = 1e-6
SCALE = HD ** -0.5

LAST_RESULT = None  # BassKernelResults of the most recent run (for test harness)


def _attention_plan(cu_seqlens):
    """Compile-time mask plan from cu_seqlens.

    Returns (plan, mask_pack):
      plan[mc] = list of (nt, w0, w1, mask_ops); w0/w1 are column offsets
      (multiples of 128, relative to the 512-wide m-chunk) of the contiguous
      valid window; mask_ops = [(j, kind, idx)] for 128-col subtiles needing
      a multiplicative 0/1 mask: kind 'tri' uses the shared causal triangle,
      kind 'host' uses mask_pack[:, idx*128:(idx+1)*128].
    """
    idx = np.arange(S)
    seg = np.searchsorted(np.asarray(cu_seqlens), idx, side="right") - 1
    mask_qk = (seg[:, None] == seg[None, :]) & (idx[:, None] >= idx[None, :])
    mask_t = mask_qk.T  # [n, m]

    plan = []
    tiles = []
    tile_ids = {}
    for mc in range(MC):
        entries = []
        for nt in range(NT):
            blk = mask_t[nt * 128:(nt + 1) * 128, mc * 512:(mc + 1) * 512]
            if not blk.any():
                continue
            js = [j for j in range(4) if blk[:, j * 128:(j + 1) * 128].any()]
            jlo, jhi = min(js), max(js)
            assert js == list(range(jlo, jhi + 1)), "valid window not contiguous"
            mops = []
            for j in range(jlo, jhi + 1):
                sub = blk[:, j * 128:(j + 1) * 128]
                if sub.all():
                    continue
                m0g = mc * 512 + j * 128
                n0g = nt * 128
                if m0g == n0g and np.array_equal(
                    sub, idx[:128][None, :] >= idx[:128][:, None]
                ):
                    mops.append((j, "tri", -1))
                else:
                    key = sub.tobytes()
                    if key not in tile_ids:
                        tile_ids[key] = len(tiles)
                        tiles.append(sub.astype(np.float32))
                    mops.append((j, "host", tile_ids[key]))
            entries.append((nt, jlo * 128, (jhi + 1) * 128, mops))
        assert entries, "every query row attends to at least itself"
        plan.append(entries)

    if tiles:
        mask_pack = np.concatenate(tiles, axis=1)
    else:
        mask_pack = np.zeros((128, 128), dtype=np.float32)
    return plan, np.ascontiguousarray(mask_pack)


def _build_graph(plan, n_mask_cols):
    import concourse.bass as bass  # noqa: PLC0415
    import concourse.mybir as mybir  # noqa: PLC0415
    import concourse.tile as tile  # noqa: PLC0415
    from concourse import bacc  # noqa: PLC0415
    from contextlib import ExitStack  # noqa: PLC0415

    f32 = mybir.dt.float32
    f32r = mybir.dt.float32r
    bf16 = mybir.dt.bfloat16
    AF = mybir.ActivationFunctionType

    nc = bacc.Bacc()
    xT_p = nc.declare_dram_parameter("xT", [D, S], bf16, isOutput=False)
    wqkv_p = nc.declare_dram_parameter("w_qkv", [128, NO * DT * 128], bf16, isOutput=False)
    wo_p = nc.declare_dram_parameter("w_o", [128, QH * D], bf16, isOutput=False)
    cs_p = nc.declare_dram_parameter("cs", [128, 4 * S], f32r, isOutput=False)
    mask_p = nc.declare_dram_parameter("mask_pack", [128, n_mask_cols], f32r, isOutput=False)
    consts_p = nc.declare_dram_parameter("consts", [128, 5 * 128], f32r, isOutput=False)
    out_p = nc.declare_dram_parameter("out", [S, D], f32, isOutput=True)

    with tile.TileContext(nc) as tc, ExitStack() as ctx:
        const = ctx.enter_context(tc.tile_pool(name="const", bufs=1))
        persist = ctx.enter_context(tc.tile_pool(name="persist", bufs=1))

        consts = const.tile([128, 5 * 128], f32r)
        nc.sync.dma_start(consts[:], consts_p[:])
        ones_col = consts[:, 0:1]
        ones_row = consts[0:1, 0:128]
        swp = consts[:, 128:256]        # swap-halves permutation
        ident = consts[:, 256:384]      # identity (for PE transpose)
        tri = consts[:, 384:512]        # causal triangle in [n, m]: 1 iff m >= n
        sca_row = consts[0:1, 512:640]  # all = HD**0.5 (divide-by folds the attn scale)

        mask_sb = const.tile([128, n_mask_cols], f32r)
        nc.sync.dma_start(mask_sb[:], mask_p[:])

        eps_col = const.tile([128, 1], f32)
        nc.gpsimd.memset(eps_col[:], EPS)

        # persistent activations: q0..q3, k, v in transposed [feat, seq] layout
        qkvT = [persist.tile([128, S], f32r, tag=f"qkvT{o}", name=f"qkvT{o}") for o in range(NO)]

        # ---------------- stage 1: qkv projection + rms stats ----------------
        with ExitStack() as s1:
            pw = s1.enter_context(tc.tile_pool(name="wqkv", bufs=1))
            px = s1.enter_context(tc.tile_pool(name="xstream", bufs=3))
            pcs = s1.enter_context(tc.tile_pool(name="csstream", bufs=2))
            psc = s1.enter_context(tc.tile_pool(name="s1scratch", bufs=2))
            pq = s1.enter_context(tc.tile_pool(name="qkvpsum", bufs=1, space="PSUM"))
            pss = s1.enter_context(tc.tile_pool(name="ssqpsum", bufs=2, space="PSUM"))

            w_sb = pw.tile([128, NO * DT * 128], bf16)
            nc.sync.dma_start(w_sb[:], wqkv_p[:])

            for mc in range(MC):
                msl = slice(mc * 512, (mc + 1) * 512)
                accs = [pq.tile([128, 512], f32, tag=f"acc{o}", name=f"acc{o}") for o in range(NO)]
                for d in range(DT):
                    xt = px.tile([128, 512], bf16, tag="xt")
                    nc.sync.dma_start(xt[:], xT_p[d * 128:(d + 1) * 128, msl])
                    for o in range(NO):
                        woff = (o * DT + d) * 128
                        nc.tensor.matmul(
                            accs[o][:],
                            w_sb[:, woff:woff + 128],
                            xt[:],
                            start=(d == 0),
                            stop=(d == DT - 1),
                        )
                for o in range(NO):
                    nc.scalar.activation(qkvT[o][:, msl], accs[o][:], AF.Copy)
                # rms stats + rope + scaling for q heads and k (v passes through)
                for o in range(QH + 1):
                    sq = psc.tile([128, 512], f32r, tag="sq")
                    nc.scalar.activation(sq[:], accs[o][:], AF.Square)
                    ss = pss.tile([1, 512], f32, tag="ssbc", name="ss", padded_shape=[128, 512])
                    nc.tensor.matmul(ss[:], ones_col, sq[:], start=True, stop=True)
                    rsq = psc.tile([1, 512], f32r, tag="rsq")
                    nc.scalar.activation(
                        rsq[:], ss[:], AF.Sqrt, bias=eps_col[0:1, :], scale=1.0 / HD
                    )

                    csb = 0 if o < QH else 2  # q heads share cs1q/cs2q; k: cs1k/cs2k
                    row = sca_row if o < QH else ones_row  # fold attn scale into q
                    cs1 = pcs.tile([128, 512], f32r, tag="cs1")
                    cs2 = pcs.tile([128, 512], f32r, tag="cs2")
                    nc.sync.dma_start(cs1[:], cs_p[:, csb * S + mc * 512: csb * S + (mc + 1) * 512])
                    nc.sync.dma_start(cs2[:], cs_p[:, (csb + 1) * S + mc * 512: (csb + 1) * S + (mc + 1) * 512])
                    # B = swap_halves(qT) via PE permutation
                    bp = pss.tile([128, 512], f32, tag="ssbc", name="bp")
                    nc.tensor.matmul(bp[:], swp, qkvT[o][:, msl], start=True, stop=True)
                    t1 = psc.tile([128, 512], f32, tag="t1")
                    nc.vector.tensor_mul(t1[:], qkvT[o][:, msl], cs1[:])
                    t2 = psc.tile([128, 512], f32, tag="t2")
                    nc.vector.tensor_mul(t2[:], bp[:], cs2[:])
                    nc.vector.tensor_add(t1[:], t1[:], t2[:])
                    # broadcast (1/scale)*sqrt(var) across partitions, invert, multiply
                    bc = pss.tile([128, 512], f32, tag="ssbc", name="bc")
                    nc.tensor.matmul(bc[:], row, rsq[:], start=True, stop=True)
                    rrb = psc.tile([128, 512], f32, tag="rrb")
                    nc.vector.reciprocal_approx_fast(out=rrb[:], in_=bc[:])
                    nc.vector.tensor_mul(qkvT[o][:, msl], t1[:], rrb[:])

        # ---------------- stage 2: attention ----------------
        with ExitStack() as s2:
            p2 = s2.enter_context(tc.tile_pool(name="persist2", bufs=1))
            v_sb = p2.tile([128, S], f32r)
            attnT = [p2.tile([128, S], bf16, tag=f"attnT{h}", name=f"attnT{h}") for h in range(QH)]
            wo_sb = p2.tile([128, QH * D], bf16)
            nc.sync.dma_start(wo_sb[:], wo_p[:])

            kT = qkvT[QH]
            vT = qkvT[QH + 1]

            with ExitStack() as s2a:
                ptp = s2a.enter_context(tc.tile_pool(name="tppsum", bufs=2, space="PSUM"))
                for nt in range(NT):
                    nsl = slice(nt * 128, (nt + 1) * 128)
                    tp = ptp.tile([128, 128], f32, tag="tp")
                    nc.tensor.transpose(
                        tp[:], vT[:, nsl].bitcast(mybir.dt.float32), ident.bitcast(mybir.dt.float32)
                    )
                    nc.scalar.activation(v_sb[:, nsl], tp[:], AF.Copy)

            with ExitStack() as s2b:
                psco = s2b.enter_context(tc.tile_pool(name="scpsum", bufs=3, space="PSUM"))
                pov = s2b.enter_context(tc.tile_pool(name="ovpsum", bufs=2, space="PSUM"))
                pden = s2b.enter_context(tc.tile_pool(name="denpsum", bufs=2, space="PSUM"))
                pbc2 = s2b.enter_context(tc.tile_pool(name="bc2psum", bufs=1, space="PSUM"))
                pex = s2b.enter_context(tc.tile_pool(name="exsbuf", bufs=3))
                pnr = s2b.enter_context(tc.tile_pool(name="nrsbuf", bufs=2))

                for h in range(QH):
                    for mc in range(MC):
                        entries = plan[mc]
                        ov = pov.tile([128, 512], f32, tag="ov")
                        den = pden.tile([1, 512], f32, tag="den")
                        n_ent = len(entries)
                        for i, (nt, w0, w1, mops) in enumerate(entries):
                            nsl = slice(nt * 128, (nt + 1) * 128)
                            qsl = slice(mc * 512 + w0, mc * 512 + w1)
                            sc = psco.tile([128, 512], f32, tag="sc")
                            nc.tensor.matmul(
                                sc[:, w0:w1], kT[:, nsl], qkvT[h][:, qsl],
                                start=True, stop=True,
                            )
                            ex = pex.tile([128, 512], f32r, tag="ex")
                            nc.scalar.activation(ex[:, w0:w1], sc[:, w0:w1], AF.Exp)
                            for (j, kind, tix) in mops:
                                jsl = slice(j * 128, (j + 1) * 128)
                                msrc = tri if kind == "tri" else mask_sb[:, tix * 128:(tix + 1) * 128]
                                nc.vector.tensor_mul(ex[:, jsl], ex[:, jsl], msrc)
                            first = i == 0
                            last = i == n_ent - 1
                            nc.tensor.matmul(
                                ov[:, w0:w1], v_sb[:, nsl], ex[:, w0:w1],
                                start=first, stop=last, skip_group_check=True,
                            )
                            nc.tensor.matmul(
                                den[0:1, w0:w1], ones_col, ex[:, w0:w1],
                                start=first, stop=last, skip_group_check=True,
                            )
                        den_sb = pnr.tile([1, 512], f32r, tag="den_sb")
                        nc.scalar.activation(den_sb[:], den[:], AF.Copy)
                        bc = pbc2.tile([128, 512], f32, tag="bc2")
                        nc.tensor.matmul(bc[:], ones_row, den_sb[:], start=True, stop=True)
                        bcs = pnr.tile([128, 512], f32, tag="bcs")
                        nc.vector.reciprocal_approx_fast(out=bcs[:], in_=bc[:])
                        nc.vector.tensor_mul(
                            attnT[h][:, mc * 512:(mc + 1) * 512], ov[:], bcs[:]
                        )

            # ---------------- stage 3: output projection ----------------
            with ExitStack() as s3:
                py = s3.enter_context(tc.tile_pool(name="ypsum", bufs=4, space="PSUM"))
                pys = s3.enter_context(tc.tile_pool(name="ysbuf", bufs=3))
                for mt in range(S // 128):
                    tsl = slice(mt * 128, (mt + 1) * 128)
                    for ec in range(D // 512):
                        yp = py.tile([128, 512], f32, tag="yp")
                        for t in range(QH):
                            nc.tensor.matmul(
                                yp[:],
                                attnT[t][:, tsl],
                                wo_sb[:, t * D + ec * 512: t * D + (ec + 1) * 512],
                                start=(t == 0),
                                stop=(t == QH - 1),
                            )
                        ys = pys.tile([128, 512], f32, tag="ys")
                        nc.scalar.activation(ys[:], yp[:], AF.Copy)
                        nc.sync.dma_start(out_p[tsl, ec * 512:(ec + 1) * 512], ys[:])

    nc.finalize()
    return nc


def kernel(x, wq, wk, wv, wo, q_norm_w, k_norm_w, rope_cache, positions, cu_seqlens):
    global LAST_RESULT
    from concourse.bass_utils import run_bass_kernel_spmd  # noqa: PLC0415

    x = np.asarray(x, dtype=np.float32)
    wq = np.asarray(wq, dtype=np.float32)
    wk = np.asarray(wk, dtype=np.float32)
    wv = np.asarray(wv, dtype=np.float32)
    wo = np.asarray(wo, dtype=np.float32)
    q_norm_w = np.asarray(q_norm_w, dtype=np.float32)
    k_norm_w = np.asarray(k_norm_w, dtype=np.float32)
    rope_cache = np.asarray(rope_cache, dtype=np.float32)
    positions = np.asarray(positions)
    cu_seqlens = np.asarray(cu_seqlens)

    import ml_dtypes  # noqa: PLC0415

    # ---- host prep (shared) ----
    xT = np.ascontiguousarray(x[0].T.astype(ml_dtypes.bfloat16))  # [D, S]

    pos = positions.reshape(-1)
    cs = rope_cache[pos]               # [S, HALF, 2]
    cosT = cs[:, :, 0].T               # [HALF, S]
    sinT = cs[:, :, 1].T
    cs1 = np.concatenate([cosT, cosT], axis=0)    # [128, S]
    cs2 = np.concatenate([-sinT, sinT], axis=0)

    def fold(w):
        w = w.reshape(HD, 1)
        wsw = np.concatenate([w[HALF:], w[:HALF]], axis=0)
        return cs1 * w, cs2 * wsw

    cs1q, cs2q = fold(q_norm_w)
    cs1k, cs2k = fold(k_norm_w)
    cs_host = np.ascontiguousarray(
        np.concatenate([cs1q, cs2q, cs1k, cs2k], axis=1), dtype=np.float32
    )  # [128, 4S]

    plan, mask_pack = _attention_plan(cu_seqlens)

    consts = np.zeros((128, 5 * 128), dtype=np.float32)
    consts[:, 0:128] = 1.0
    swp = np.zeros((128, 128), dtype=np.float32)
    swp[np.arange(128), (np.arange(128) + HALF) % 128] = 1.0
    consts[:, 128:256] = swp
    consts[:, 256:384] = np.eye(128, dtype=np.float32)
    consts[:, 384:512] = np.triu(np.ones((128, 128), dtype=np.float32))
    consts[:, 512:640] = 1.0 / SCALE

    # ---- per-core weight shards ----
    in_maps = []
    for c in range(NCORES):
        w_all = np.concatenate(
            [
                wq[c * QH * HD:(c + 1) * QH * HD],   # [512, D]
                wk[c * HD:(c + 1) * HD],             # [128, D]
                wv[c * HD:(c + 1) * HD],             # [128, D]
            ],
            axis=0,
        )  # [NO*128, D]
        w_host = np.ascontiguousarray(
            w_all.reshape(NO, 128, DT, 128).transpose(3, 0, 2, 1)
            .reshape(128, NO * DT * 128).astype(ml_dtypes.bfloat16)
        )
        wo_c = wo[:, c * QH * HD:(c + 1) * QH * HD].T  # [512, D]
        wo_host = np.ascontiguousarray(
            wo_c.reshape(QH, 128, D).transpose(1, 0, 2)
            .reshape(128, QH * D).astype(ml_dtypes.bfloat16)
        )
        in_maps.append(
            {
                "xT": xT,
                "w_qkv": w_host,
                "w_o": wo_host,
                "cs": cs_host,
                "mask_pack": mask_pack,
                "consts": consts,
            }
        )

    nc = _build_graph(plan, mask_pack.shape[1])
    res = run_bass_kernel_spmd(nc, in_maps, list(range(NCORES)))
    LAST_RESULT = res

    out = res.results[0]["out"].astype(np.float32)
    for c in range(1, NCORES):
        out = out + res.results[c]["out"]
    return out.reshape(1, S, D)


# revision 12
# speedup vs baseline: 1.4240x; 1.0994x over previous
"""Trainium2 Bass kernel for nn_Attention_2216203124924 (sparse/varlen GQA attention).

Full computation:
  xq/xk/xv = x @ {wq,wk,wv}.T ; per-head RMSNorm(q,k) ; RoPE via
  rope_cache[positions] ; GQA repeat ; per-segment causal attention
  (segments from cu_seqlens) ; out @ wo.T

Distribution (8 NeuronCores, tensor-parallel over heads):
  core c owns q-heads [4c,4c+4) and kv-head c (GQA groups align),
  wo is row-sharded; each core emits a partial [2048,4096] output and the
  host sums the 8 partials.

On-device layout is "transposed" ([feature, seq]) throughout so the
contraction dim always sits on SBUF partitions:
  qT/kT/vT from weight-stationary projection matmuls, RMSNorm stats via
  ones-column matmul + matmul-broadcast of rsqrt row, RoPE as elementwise
  muls with host-gathered cos/sin (+ PE swap-half permutation), scores^T =
  kT_tile.T @ qT, probs via unnormalized exp (scores are O(1), max-sub
  unneeded) with compile-time segment mask plan, PV accumulated over key
  tiles in PSUM, normalization by matmul-broadcast reciprocal row, and the
  output projection from attnT tiles against wo^T.

All matmul operands are float32r (~13-bit mantissa, full PE rate).
The segment/causal structure from cu_seqlens and the rope gather by
positions are resolved on the host at build time; the NEFF is specialized
to them.
"""

import os
import sys

import numpy as np

for _p in ("/opt/trn_rl_repo",):
    if os.path.isdir(_p) and _p not in sys.path:
        sys.path.insert(0, _p)

S = 2048
D = 4096
HD = 128
HALF = 64
N_HEADS = 32
N_KV = 8
NCORES = 8
QH = N_HEADS // NCORES          # 4 q heads per core
NO = QH + 2                     # o-tiles per core in qkv projection: q0..q3, k, v
DT = D // 128                   # 32 contraction tiles
MC = S // 512                   # 4 m-chunks of 512
NT = S // 128                   # 16 key tiles
EPS = 1e-6
SCALE = HD ** -0.5

LAST_RESULT = None  # BassKernelResults of the most recent run (for test harness)


def _attention_plan(cu_seqlens):
    """Compile-time mask plan from cu_seqlens.

    Returns (plan, mask_pack):
      plan[mc] = list of (nt, w0, w1, mask_ops); w0/w1 are column offsets
      (multiples of 128, relative to the 512-wide m-chunk) of the contiguous
      valid window; mask_ops = [(j, kind, idx)] for 128-col subtiles needing
      a multiplicative 0/1 mask: kind 'tri' uses the shared causal triangle,
      kind 'host' uses mask_pack[:, idx*128:(idx+1)*128].
    """
    idx = np.arange(S)
    seg = np.searchsorted(np.asarray(cu_seqlens), idx, side="right") - 1
    mask_qk = (seg[:, None] == seg[None, :]) & (idx[:, None] >= idx[None, :])
    mask_t = mask_qk.T  # [n, m]

    plan = []
    tiles = []
    tile_ids = {}
    for mc in range(MC):
        entries = []
        for nt in range(NT):
            blk = mask_t[nt * 128:(nt + 1) * 128, mc * 512:(mc + 1) * 512]
            if not blk.any():
                continue
            js = [j for j in range(4) if blk[:, j * 128:(j + 1) * 128].any()]
            jlo, jhi = min(js), max(js)
            assert js == list(range(jlo, jhi + 1)), "valid window not contiguous"
            mops = []
            for j in range(jlo, jhi + 1):
                sub = blk[:, j * 128:(j + 1) * 128]
                if sub.all():
                    continue
                m0g = mc * 512 + j * 128
                n0g = nt * 128
                if m0g == n0g and np.array_equal(
                    sub, idx[:128][None, :] >= idx[:128][:, None]
                ):
                    mops.append((j, "tri", -1))
                else:
                    key = sub.tobytes()
                    if key not in tile_ids:
                        tile_ids[key] = len(tiles)
                        tiles.append(sub.astype(np.float32))
                    mops.append((j, "host", tile_ids[key]))
            entries.append((nt, jlo * 128, (jhi + 1) * 128, mops))
        assert entries, "every query row attends to at least itself"
        plan.append(entries)

    if tiles:
        mask_pack = np.concatenate(tiles, axis=1)
    else:
        mask_pack = np.zeros((128, 128), dtype=np.float32)
    return plan, np.ascontiguousarray(mask_pack)


def _build_graph(plan, n_mask_cols):
    import concourse.bass as bass  # noqa: PLC0415
    import concourse.mybir as mybir  # noqa: PLC0415
    import concourse.tile as tile  # noqa: PLC0415
    from concourse import bacc  # noqa: PLC0415
    from contextlib import ExitStack  # noqa: PLC0415

    f32 = mybir.dt.float32
    f32r = mybir.dt.float32r
    bf16 = mybir.dt.bfloat16
    AF = mybir.ActivationFunctionType

    nc = bacc.Bacc()
    xT_p = nc.declare_dram_parameter("xT", [D, S], bf16, isOutput=False)
    wqkv_p = nc.declare_dram_parameter("w_qkv", [128, NO * DT * 128], bf16, isOutput=False)
    wo_p = nc.declare_dram_parameter("w_o", [128, QH * D], bf16, isOutput=False)
    cs_p = nc.declare_dram_parameter("cs", [128, 4 * S], f32r, isOutput=False)
    mask_p = nc.declare_dram_parameter("mask_pack", [128, n_mask_cols], f32r, isOutput=False)
    consts_p = nc.declare_dram_parameter("consts", [128, 5 * 128], f32r, isOutput=False)
    out_p = nc.declare_dram_parameter("out", [S, D], f32, isOutput=True)

    with tile.TileContext(nc) as tc, ExitStack() as ctx:
        const = ctx.enter_context(tc.tile_pool(name="const", bufs=1))
        persist = ctx.enter_context(tc.tile_pool(name="persist", bufs=1))

        consts = const.tile([128, 5 * 128], f32r)
        nc.sync.dma_start(consts[:], consts_p[:])
        ones_col = consts[:, 0:1]
        ones_row = consts[0:1, 0:128]
        swp = consts[:, 128:256]        # swap-halves permutation
        ident = consts[:, 256:384]      # identity (for PE transpose)
        tri = consts[:, 384:512]        # causal triangle in [n, m]: 1 iff m >= n
        sca_row = consts[0:1, 512:640]  # all = HD**0.5 (divide-by folds the attn scale)

        mask_sb = const.tile([128, n_mask_cols], f32r)
        nc.sync.dma_start(mask_sb[:], mask_p[:])

        eps_col = const.tile([128, 1], f32)
        nc.gpsimd.memset(eps_col[:], EPS)

        # persistent activations: q0..q3, k, v in transposed [feat, seq] layout
        qkvT = [persist.tile([128, S], f32r, tag=f"qkvT{o}", name=f"qkvT{o}") for o in range(NO)]

        # ---------------- stage 1: qkv projection + rms stats ----------------
        with ExitStack() as s1:
            pw = s1.enter_context(tc.tile_pool(name="wqkv", bufs=1))
            px = s1.enter_context(tc.tile_pool(name="xstream", bufs=3))
            pcs = s1.enter_context(tc.tile_pool(name="csstream", bufs=2))
            psc = s1.enter_context(tc.tile_pool(name="s1scratch", bufs=2))
            pq = s1.enter_context(tc.tile_pool(name="qkvpsum", bufs=1, space="PSUM"))
            pss = s1.enter_context(tc.tile_pool(name="ssqpsum", bufs=2, space="PSUM"))

            w_sb = pw.tile([128, NO * DT * 128], bf16)
            wchunk = DT // 4 * NO * 128
            for wci in range(4):
                nc.sync.dma_start(
                    w_sb[:, wci * wchunk:(wci + 1) * wchunk],
                    wqkv_p[:, wci * wchunk:(wci + 1) * wchunk],
                )

            for mc in range(MC):
                msl = slice(mc * 512, (mc + 1) * 512)
                accs = [pq.tile([128, 512], f32, tag=f"acc{o}", name=f"acc{o}") for o in range(NO)]
                for d in range(DT):
                    xt = px.tile([128, 512], bf16, tag="xt")
                    nc.sync.dma_start(xt[:], xT_p[d * 128:(d + 1) * 128, msl])
                    for o in range(NO):
                        woff = (d * NO + o) * 128
                        nc.tensor.matmul(
                            accs[o][:],
                            w_sb[:, woff:woff + 128],
                            xt[:],
                            start=(d == 0),
                            stop=(d == DT - 1),
                        )
                for o in range(NO):
                    nc.vector.tensor_copy(qkvT[o][:, msl], accs[o][:])
                # rms stats + rope + scaling for q heads and k (v passes through)
                for o in range(QH + 1):
                    sq = psc.tile([128, 512], f32r, tag="sq")
                    nc.scalar.activation(sq[:], qkvT[o][:, msl], AF.Square)
                    ss = pss.tile([1, 512], f32, tag="ssbc", name="ss", padded_shape=[128, 512])
                    nc.tensor.matmul(ss[:], ones_col, sq[:], start=True, stop=True)
                    rsq = psc.tile([1, 512], f32r, tag="rsq")
                    nc.scalar.activation(
                        rsq[:], ss[:], AF.Sqrt, bias=eps_col[0:1, :], scale=1.0 / HD
                    )

                    csb = 0 if o < QH else 2  # q heads share cs1q/cs2q; k: cs1k/cs2k
                    row = sca_row if o < QH else ones_row  # fold attn scale into q
                    cs1 = pcs.tile([128, 512], f32r, tag="cs1")
                    cs2 = pcs.tile([128, 512], f32r, tag="cs2")
                    nc.sync.dma_start(cs1[:], cs_p[:, csb * S + mc * 512: csb * S + (mc + 1) * 512])
                    nc.sync.dma_start(cs2[:], cs_p[:, (csb + 1) * S + mc * 512: (csb + 1) * S + (mc + 1) * 512])
                    # B = swap_halves(qT) via PE permutation
                    bp = pss.tile([128, 512], f32, tag="ssbc", name="bp")
                    nc.tensor.matmul(bp[:], swp, qkvT[o][:, msl], start=True, stop=True)
                    t1 = psc.tile([128, 512], f32, tag="t1")
                    nc.vector.tensor_mul(t1[:], qkvT[o][:, msl], cs1[:])
                    t2 = psc.tile([128, 512], f32, tag="t2")
                    nc.vector.tensor_mul(t2[:], bp[:], cs2[:])
                    nc.vector.tensor_add(t1[:], t1[:], t2[:])
                    # broadcast (1/scale)*sqrt(var) across partitions, invert, multiply
                    bc = pss.tile([128, 512], f32, tag="ssbc", name="bc")
                    nc.tensor.matmul(bc[:], row, rsq[:], start=True, stop=True)
                    rrb = psc.tile([128, 512], f32, tag="rrb")
                    nc.vector.reciprocal_approx_fast(out=rrb[:], in_=bc[:])
                    nc.vector.tensor_mul(qkvT[o][:, msl], t1[:], rrb[:])

        # ---------------- stage 2: attention ----------------
        with ExitStack() as s2:
            p2 = s2.enter_context(tc.tile_pool(name="persist2", bufs=1))
            v_sb = p2.tile([128, S], f32r)
            attnT = [p2.tile([128, S], bf16, tag=f"attnT{h}", name=f"attnT{h}") for h in range(QH)]
            wo_sb = p2.tile([128, QH * D], bf16)
            nc.sync.dma_start(wo_sb[:], wo_p[:])

            kT = qkvT[QH]
            vT = qkvT[QH + 1]

            with ExitStack() as s2a:
                ptp = s2a.enter_context(tc.tile_pool(name="tppsum", bufs=2, space="PSUM"))
                for nt in range(NT):
                    nsl = slice(nt * 128, (nt + 1) * 128)
                    tp = ptp.tile([128, 128], f32, tag="tp")
                    nc.tensor.transpose(
                        tp[:], vT[:, nsl].bitcast(mybir.dt.float32), ident.bitcast(mybir.dt.float32)
                    )
                    nc.vector.tensor_copy(v_sb[:, nsl], tp[:])

            with ExitStack() as s2b:
                psco = s2b.enter_context(tc.tile_pool(name="scpsum", bufs=3, space="PSUM"))
                pov = s2b.enter_context(tc.tile_pool(name="ovpsum", bufs=2, space="PSUM"))
                pden = s2b.enter_context(tc.tile_pool(name="denpsum", bufs=2, space="PSUM"))
                pbc2 = s2b.enter_context(tc.tile_pool(name="bc2psum", bufs=1, space="PSUM"))
                pex = s2b.enter_context(tc.tile_pool(name="exsbuf", bufs=3))
                pnr = s2b.enter_context(tc.tile_pool(name="nrsbuf", bufs=2))

                for h in range(QH):
                    for mc in range(MC):
                        entries = plan[mc]
                        ov = pov.tile([128, 512], f32, tag="ov")
                        den = pden.tile([1, 512], f32, tag="den")
                        n_ent = len(entries)
                        for i, (nt, w0, w1, mops) in enumerate(entries):
                            nsl = slice(nt * 128, (nt + 1) * 128)
                            qsl = slice(mc * 512 + w0, mc * 512 + w1)
                            sc = psco.tile([128, 512], f32, tag="sc")
                            nc.tensor.matmul(
                                sc[:, w0:w1], kT[:, nsl], qkvT[h][:, qsl],
                                start=True, stop=True,
                            )
                            ex = pex.tile([128, 512], f32r, tag="ex")
                            nc.scalar.activation(ex[:, w0:w1], sc[:, w0:w1], AF.Exp)
                            for (j, kind, tix) in mops:
                                jsl = slice(j * 128, (j + 1) * 128)
                                msrc = tri if kind == "tri" else mask_sb[:, tix * 128:(tix + 1) * 128]
                                nc.vector.tensor_mul(ex[:, jsl], ex[:, jsl], msrc)
                            first = i == 0
                            last = i == n_ent - 1
                            nc.tensor.matmul(
                                ov[:, w0:w1], v_sb[:, nsl], ex[:, w0:w1],
                                start=first, stop=last, skip_group_check=True,
                            )
                            nc.tensor.matmul(
                                den[0:1, w0:w1], ones_col, ex[:, w0:w1],
                                start=first, stop=last, skip_group_check=True,
                            )
                        den_sb = pnr.tile([1, 512], f32r, tag="den_sb")
                        nc.scalar.activation(den_sb[:], den[:], AF.Copy)
                        bc = pbc2.tile([128, 512], f32, tag="bc2")
                        nc.tensor.matmul(bc[:], ones_row, den_sb[:], start=True, stop=True)
                        bcs = pnr.tile([128, 512], f32, tag="bcs")
                        nc.vector.reciprocal_approx_fast(out=bcs[:], in_=bc[:])
                        nc.vector.tensor_mul(
                            attnT[h][:, mc * 512:(mc + 1) * 512], ov[:], bcs[:]
                        )

            # ---------------- stage 3: output projection ----------------
            with ExitStack() as s3:
                py = s3.enter_context(tc.tile_pool(name="ypsum", bufs=4, space="PSUM"))
                pys = s3.enter_context(tc.tile_pool(name="ysbuf", bufs=3))
                for mt in range(S // 128):
                    tsl = slice(mt * 128, (mt + 1) * 128)
                    for ec in range(D // 512):
                        yp = py.tile([128, 512], f32, tag="yp")
                        for t in range(QH):
                            nc.tensor.matmul(
                                yp[:],
                                attnT[t][:, tsl],
                                wo_sb[:, t * D + ec * 512: t * D + (ec + 1) * 512],
                                start=(t == 0),
                                stop=(t == QH - 1),
                            )
                        ys = pys.tile([128, 512], f32, tag="ys")
                        nc.scalar.activation(ys[:], yp[:], AF.Copy)
                        nc.sync.dma_start(out_p[tsl, ec * 512:(ec + 1) * 512], ys[:])

    nc.finalize()
    return nc


def kernel(x, wq, wk, wv, wo, q_norm_w, k_norm_w, rope_cache, positions, cu_seqlens):
    global LAST_RESULT
    from concourse.bass_utils import run_bass_kernel_spmd  # noqa: PLC0415

    x = np.asarray(x, dtype=np.float32)
    wq = np.asarray(wq, dtype=np.float32)
    wk = np.asarray(wk, dtype=np.float32)
    wv = np.asarray(wv, dtype=np.float32)
    wo = np.asarray(wo, dtype=np.float32)
    q_norm_w = np.asarray(q_norm_w, dtype=np.float32)
    k_norm_w = np.asarray(k_norm_w, dtype=np.float32)
    rope_cache = np.asarray(rope_cache, dtype=np.float32)
    positions = np.asarray(positions)
    cu_seqlens = np.asarray(cu_seqlens)

    import ml_dtypes  # noqa: PLC0415

    # ---- host prep (shared) ----
    xT = np.ascontiguousarray(x[0].T.astype(ml_dtypes.bfloat16))  # [D, S]

    pos = positions.reshape(-1)
    cs = rope_cache[pos]               # [S, HALF, 2]
    cosT = cs[:, :, 0].T               # [HALF, S]
    sinT = cs[:, :, 1].T
    cs1 = np.concatenate([cosT, cosT], axis=0)    # [128, S]
    cs2 = np.concatenate([-sinT, sinT], axis=0)

    def fold(w):
        w = w.reshape(HD, 1)
        wsw = np.concatenate([w[HALF:], w[:HALF]], axis=0)
        return cs1 * w, cs2 * wsw

    cs1q, cs2q = fold(q_norm_w)
    cs1k, cs2k = fold(k_norm_w)
    cs_host = np.ascontiguousarray(
        np.concatenate([cs1q, cs2q, cs1k, cs2k], axis=1), dtype=np.float32
    )  # [128, 4S]

    plan, mask_pack = _attention_plan(cu_seqlens)

    consts = np.zeros((128, 5 * 128), dtype=np.float32)
    consts[:, 0:128] = 1.0
    swp = np.zeros((128, 128), dtype=np.float32)
    swp[np.arange(128), (np.arange(128) + HALF) % 128] = 1.0
    consts[:, 128:256] = swp
    consts[:, 256:384] = np.eye(128, dtype=np.float32)
    consts[:, 384:512] = np.triu(np.ones((128, 128), dtype=np.float32))
    consts[:, 512:640] = 1.0 / SCALE

    # ---- per-core weight shards ----
    in_maps = []
    for c in range(NCORES):
        w_all = np.concatenate(
            [
                wq[c * QH * HD:(c + 1) * QH * HD],   # [512, D]
                wk[c * HD:(c + 1) * HD],             # [128, D]
                wv[c * HD:(c + 1) * HD],             # [128, D]
            ],
            axis=0,
        )  # [NO*128, D]
        w_host = np.ascontiguousarray(
            w_all.reshape(NO, 128, DT, 128).transpose(3, 2, 0, 1)
            .reshape(128, NO * DT * 128).astype(ml_dtypes.bfloat16)
        )
        wo_c = wo[:, c * QH * HD:(c + 1) * QH * HD].T  # [512, D]
        wo_host = np.ascontiguousarray(
            wo_c.reshape(QH, 128, D).transpose(1, 0, 2)
            .reshape(128, QH * D).astype(ml_dtypes.bfloat16)
        )
        in_maps.append(
            {
                "xT": xT,
                "w_qkv": w_host,
                "w_o": wo_host,
                "cs": cs_host,
                "mask_pack": mask_pack,
                "consts": consts,
            }
        )

    nc = _build_graph(plan, mask_pack.shape[1])
    res = run_bass_kernel_spmd(nc, in_maps, list(range(NCORES)))
    LAST_RESULT = res

    out = res.results[0]["out"].astype(np.float32)
    for c in range(1, NCORES):
        out = out + res.results[c]["out"]
    return out.reshape(1, S, D)


# revision 13
# speedup vs baseline: 1.4287x; 1.0033x over previous
"""Trainium2 Bass kernel for nn_Attention_2216203124924 (sparse/varlen GQA attention).

Full computation:
  xq/xk/xv = x @ {wq,wk,wv}.T ; per-head RMSNorm(q,k) ; RoPE via
  rope_cache[positions] ; GQA repeat ; per-segment causal attention
  (segments from cu_seqlens) ; out @ wo.T

Distribution (8 NeuronCores, tensor-parallel over heads):
  core c owns q-heads [4c,4c+4) and kv-head c (GQA groups align),
  wo is row-sharded; each core emits a partial [2048,4096] output and the
  host sums the 8 partials.

On-device layout is "transposed" ([feature, seq]) throughout so the
contraction dim always sits on SBUF partitions:
  qT/kT/vT from weight-stationary projection matmuls, RMSNorm stats via
  ones-column matmul + matmul-broadcast of rsqrt row, RoPE as elementwise
  muls with host-gathered cos/sin (+ PE swap-half permutation), scores^T =
  kT_tile.T @ qT, probs via unnormalized exp (scores are O(1), max-sub
  unneeded) with compile-time segment mask plan, PV accumulated over key
  tiles in PSUM, normalization by matmul-broadcast reciprocal row, and the
  output projection from attnT tiles against wo^T.

All matmul operands are float32r (~13-bit mantissa, full PE rate).
The segment/causal structure from cu_seqlens and the rope gather by
positions are resolved on the host at build time; the NEFF is specialized
to them.
"""

import os
import sys

import numpy as np

for _p in ("/opt/trn_rl_repo",):
    if os.path.isdir(_p) and _p not in sys.path:
        sys.path.insert(0, _p)

S = 2048
D = 4096
HD = 128
HALF = 64
N_HEADS = 32
N_KV = 8
NCORES = 8
QH = N_HEADS // NCORES          # 4 q heads per core
NO = QH + 2                     # o-tiles per core in qkv projection: q0..q3, k, v
DT = D // 128                   # 32 contraction tiles
MC = S // 512                   # 4 m-chunks of 512
NT = S // 128                   # 16 key tiles
EPS = 1e-6
SCALE = HD ** -0.5

LAST_RESULT = None  # BassKernelResults of the most recent run (for test harness)


def _attention_plan(cu_seqlens):
    """Compile-time mask plan from cu_seqlens.

    Returns (plan, mask_pack):
      plan[mc] = list of (nt, w0, w1, mask_ops); w0/w1 are column offsets
      (multiples of 128, relative to the 512-wide m-chunk) of the contiguous
      valid window; mask_ops = [(j, kind, idx)] for 128-col subtiles needing
      a multiplicative 0/1 mask: kind 'tri' uses the shared causal triangle,
      kind 'host' uses mask_pack[:, idx*128:(idx+1)*128].
    """
    idx = np.arange(S)
    seg = np.searchsorted(np.asarray(cu_seqlens), idx, side="right") - 1
    mask_qk = (seg[:, None] == seg[None, :]) & (idx[:, None] >= idx[None, :])
    mask_t = mask_qk.T  # [n, m]

    plan = []
    tiles = []
    tile_ids = {}
    for mc in range(MC):
        entries = []
        for nt in range(NT):
            blk = mask_t[nt * 128:(nt + 1) * 128, mc * 512:(mc + 1) * 512]
            if not blk.any():
                continue
            js = [j for j in range(4) if blk[:, j * 128:(j + 1) * 128].any()]
            jlo, jhi = min(js), max(js)
            assert js == list(range(jlo, jhi + 1)), "valid window not contiguous"
            mops = []
            for j in range(jlo, jhi + 1):
                sub = blk[:, j * 128:(j + 1) * 128]
                if sub.all():
                    continue
                m0g = mc * 512 + j * 128
                n0g = nt * 128
                if m0g == n0g and np.array_equal(
                    sub, idx[:128][None, :] >= idx[:128][:, None]
                ):
                    mops.append((j, "tri", -1))
                else:
                    key = sub.tobytes()
                    if key not in tile_ids:
                        tile_ids[key] = len(tiles)
                        tiles.append(sub.astype(np.float32))
                    mops.append((j, "host", tile_ids[key]))
            entries.append((nt, jlo * 128, (jhi + 1) * 128, mops))
        assert entries, "every query row attends to at least itself"
        plan.append(entries)

    if tiles:
        mask_pack = np.concatenate(tiles, axis=1)
    else:
        mask_pack = np.zeros((128, 128), dtype=np.float32)
    return plan, np.ascontiguousarray(mask_pack)


def _build_graph(plan, n_mask_cols):
    import concourse.bass as bass  # noqa: PLC0415
    import concourse.mybir as mybir  # noqa: PLC0415
    import concourse.tile as tile  # noqa: PLC0415
    from concourse import bacc  # noqa: PLC0415
    from contextlib import ExitStack  # noqa: PLC0415

    f32 = mybir.dt.float32
    f32r = mybir.dt.float32r
    bf16 = mybir.dt.bfloat16
    AF = mybir.ActivationFunctionType

    nc = bacc.Bacc()
    xT_p = nc.declare_dram_parameter("xT", [D, S], bf16, isOutput=False)
    wqkv_p = nc.declare_dram_parameter("w_qkv", [128, NO * DT * 128], bf16, isOutput=False)
    wo_p = nc.declare_dram_parameter("w_o", [128, QH * D], bf16, isOutput=False)
    cs_p = nc.declare_dram_parameter("cs", [128, 4 * S], f32r, isOutput=False)
    mask_p = nc.declare_dram_parameter("mask_pack", [128, n_mask_cols], f32r, isOutput=False)
    consts_p = nc.declare_dram_parameter("consts", [128, 5 * 128], f32r, isOutput=False)
    out_p = nc.declare_dram_parameter("out", [S, D], f32, isOutput=True)

    with tile.TileContext(nc) as tc, ExitStack() as ctx:
        const = ctx.enter_context(tc.tile_pool(name="const", bufs=1))
        persist = ctx.enter_context(tc.tile_pool(name="persist", bufs=1))

        consts = const.tile([128, 5 * 128], f32r)
        nc.sync.dma_start(consts[:], consts_p[:])
        ones_col = consts[:, 0:1]
        ones_row = consts[0:1, 0:128]
        swp = consts[:, 128:256]        # swap-halves permutation
        ident = consts[:, 256:384]      # identity (for PE transpose)
        tri = consts[:, 384:512]        # causal triangle in [n, m]: 1 iff m >= n
        sca_row = consts[0:1, 512:640]  # all = HD**0.5 (divide-by folds the attn scale)

        mask_sb = const.tile([128, n_mask_cols], f32r)
        nc.sync.dma_start(mask_sb[:], mask_p[:])

        eps_col = const.tile([128, 1], f32)
        nc.gpsimd.memset(eps_col[:], EPS)

        # persistent activations: q0..q3, k, v in transposed [feat, seq] layout
        qkvT = [persist.tile([128, S], f32r, tag=f"qkvT{o}", name=f"qkvT{o}") for o in range(NO)]

        # ---------------- stage 1: qkv projection + rms stats ----------------
        with ExitStack() as s1:
            pw = s1.enter_context(tc.tile_pool(name="wqkv", bufs=1))
            px = s1.enter_context(tc.tile_pool(name="xstream", bufs=6))
            pq = s1.enter_context(tc.tile_pool(name="qkvpsum", bufs=1, space="PSUM"))

            w_sb = pw.tile([128, NO * DT * 128], bf16)
            wchunk = DT // 4 * NO * 128
            for wci in range(4):
                nc.sync.dma_start(
                    w_sb[:, wci * wchunk:(wci + 1) * wchunk],
                    wqkv_p[:, wci * wchunk:(wci + 1) * wchunk],
                )

            for mc in range(MC):
                msl = slice(mc * 512, (mc + 1) * 512)
                accs = [pq.tile([128, 512], f32, tag=f"acc{o}", name=f"acc{o}") for o in range(NO)]
                for d in range(DT):
                    xt = px.tile([128, 512], bf16, tag="xt")
                    nc.sync.dma_start(xt[:], xT_p[d * 128:(d + 1) * 128, msl])
                    for o in range(NO):
                        woff = (d * NO + o) * 128
                        nc.tensor.matmul(
                            accs[o][:],
                            w_sb[:, woff:woff + 128],
                            xt[:],
                            start=(d == 0),
                            stop=(d == DT - 1),
                        )
                for o in range(NO):
                    nc.vector.tensor_copy(qkvT[o][:, msl], accs[o][:])

        # ---------------- stage 2: rope + attention, interleaved per head ----------------
        with ExitStack() as s2:
            p2 = s2.enter_context(tc.tile_pool(name="persist2", bufs=1))
            v_sb = p2.tile([128, S], f32r)
            attnT = [p2.tile([128, S], bf16, tag=f"attnT{h}", name=f"attnT{h}") for h in range(QH)]
            wo_sb = p2.tile([128, QH * D], bf16)
            nc.sync.dma_start(wo_sb[:], wo_p[:])

            kT = qkvT[QH]
            vT = qkvT[QH + 1]

            with ExitStack() as s2b:
                pcs = s2b.enter_context(tc.tile_pool(name="csstream", bufs=2))
                psc = s2b.enter_context(tc.tile_pool(name="s2scratch", bufs=2))
                pss = s2b.enter_context(tc.tile_pool(name="ssqpsum", bufs=2, space="PSUM"))
                psco = s2b.enter_context(tc.tile_pool(name="scpsum", bufs=2, space="PSUM"))
                pov = s2b.enter_context(tc.tile_pool(name="ovpsum", bufs=2, space="PSUM"))
                pden = s2b.enter_context(tc.tile_pool(name="denpsum", bufs=1, space="PSUM"))
                pbc2 = s2b.enter_context(tc.tile_pool(name="bc2psum", bufs=1, space="PSUM"))
                pex = s2b.enter_context(tc.tile_pool(name="exsbuf", bufs=3))
                pnr = s2b.enter_context(tc.tile_pool(name="nrsbuf", bufs=2))

                def rope_chain(o):
                    csb = 0 if o < QH else 2
                    row = sca_row if o < QH else ones_row
                    for mc in range(MC):
                        msl = slice(mc * 512, (mc + 1) * 512)
                        sq = psc.tile([128, 512], f32r, tag="sq", name="sq")
                        nc.scalar.activation(sq[:], qkvT[o][:, msl], AF.Square)
                        ss = pss.tile([1, 512], f32, tag="ssbc", name="ss", padded_shape=[128, 512])
                        nc.tensor.matmul(ss[:], ones_col, sq[:], start=True, stop=True)
                        rsq = psc.tile([1, 512], f32r, tag="rsq", name="rsq")
                        nc.scalar.activation(
                            rsq[:], ss[:], AF.Sqrt, bias=eps_col[0:1, :], scale=1.0 / HD
                        )
                        cs1 = pcs.tile([128, 512], f32r, tag="cs1", name="cs1")
                        cs2 = pcs.tile([128, 512], f32r, tag="cs2", name="cs2")
                        nc.sync.dma_start(cs1[:], cs_p[:, csb * S + mc * 512: csb * S + (mc + 1) * 512])
                        nc.sync.dma_start(cs2[:], cs_p[:, (csb + 1) * S + mc * 512: (csb + 1) * S + (mc + 1) * 512])
                        bp = pss.tile([128, 512], f32, tag="ssbc", name="bp")
                        nc.tensor.matmul(bp[:], swp, qkvT[o][:, msl], start=True, stop=True)
                        t1 = psc.tile([128, 512], f32, tag="t1", name="t1")
                        nc.vector.tensor_mul(t1[:], qkvT[o][:, msl], cs1[:])
                        t2 = psc.tile([128, 512], f32, tag="t2", name="t2")
                        nc.vector.tensor_mul(t2[:], bp[:], cs2[:])
                        nc.vector.tensor_add(t1[:], t1[:], t2[:])
                        bc = pss.tile([128, 512], f32, tag="ssbc", name="bc")
                        nc.tensor.matmul(bc[:], row, rsq[:], start=True, stop=True)
                        rrb = psc.tile([128, 512], f32, tag="rrb", name="rrb")
                        nc.vector.reciprocal_approx_fast(out=rrb[:], in_=bc[:])
                        nc.vector.tensor_mul(qkvT[o][:, msl], t1[:], rrb[:])

                # k first, then v transposes, then each q head followed by its attention
                rope_chain(QH)
                for nt in range(NT):
                    nsl = slice(nt * 128, (nt + 1) * 128)
                    tp = pss.tile([128, 128], f32, tag="ssbc", name="tp")
                    nc.tensor.transpose(
                        tp[:], vT[:, nsl].bitcast(mybir.dt.float32), ident.bitcast(mybir.dt.float32)
                    )
                    nc.vector.tensor_copy(v_sb[:, nsl], tp[:])

                for h in range(QH):
                    rope_chain(h)
                    for mc in range(MC):
                        entries = plan[mc]
                        ov = pov.tile([128, 512], f32, tag="ov")
                        den = pden.tile([1, 512], f32, tag="den")
                        n_ent = len(entries)
                        for i, (nt, w0, w1, mops) in enumerate(entries):
                            nsl = slice(nt * 128, (nt + 1) * 128)
                            qsl = slice(mc * 512 + w0, mc * 512 + w1)
                            sc = psco.tile([128, 512], f32, tag="sc")
                            nc.tensor.matmul(
                                sc[:, w0:w1], kT[:, nsl], qkvT[h][:, qsl],
                                start=True, stop=True,
                            )
                            ex = pex.tile([128, 512], f32r, tag="ex")
                            nc.scalar.activation(ex[:, w0:w1], sc[:, w0:w1], AF.Exp)
                            for (j, kind, tix) in mops:
                                jsl = slice(j * 128, (j + 1) * 128)
                                msrc = tri if kind == "tri" else mask_sb[:, tix * 128:(tix + 1) * 128]
                                nc.vector.tensor_mul(ex[:, jsl], ex[:, jsl], msrc)
                            first = i == 0
                            last = i == n_ent - 1
                            nc.tensor.matmul(
                                ov[:, w0:w1], v_sb[:, nsl], ex[:, w0:w1],
                                start=first, stop=last, skip_group_check=True,
                            )
                            nc.tensor.matmul(
                                den[0:1, w0:w1], ones_col, ex[:, w0:w1],
                                start=first, stop=last, skip_group_check=True,
                            )
                        den_sb = pnr.tile([1, 512], f32r, tag="den_sb")
                        nc.scalar.activation(den_sb[:], den[:], AF.Copy)
                        bc = pbc2.tile([128, 512], f32, tag="bc2")
                        nc.tensor.matmul(bc[:], ones_row, den_sb[:], start=True, stop=True)
                        bcs = pnr.tile([128, 512], f32, tag="bcs")
                        nc.vector.reciprocal_approx_fast(out=bcs[:], in_=bc[:])
                        nc.vector.tensor_mul(
                            attnT[h][:, mc * 512:(mc + 1) * 512], ov[:], bcs[:]
                        )

            # ---------------- stage 3: output projection ----------------
            with ExitStack() as s3:
                py = s3.enter_context(tc.tile_pool(name="ypsum", bufs=4, space="PSUM"))
                pys = s3.enter_context(tc.tile_pool(name="ysbuf", bufs=3))
                for mt in range(S // 128):
                    tsl = slice(mt * 128, (mt + 1) * 128)
                    for ec in range(D // 512):
                        yp = py.tile([128, 512], f32, tag="yp")
                        for t in range(QH):
                            nc.tensor.matmul(
                                yp[:],
                                attnT[t][:, tsl],
                                wo_sb[:, t * D + ec * 512: t * D + (ec + 1) * 512],
                                start=(t == 0),
                                stop=(t == QH - 1),
                            )
                        ys = pys.tile([128, 512], f32, tag="ys")
                        nc.scalar.activation(ys[:], yp[:], AF.Copy)
                        nc.sync.dma_start(out_p[tsl, ec * 512:(ec + 1) * 512], ys[:])

    nc.finalize()
    return nc


def kernel(x, wq, wk, wv, wo, q_norm_w, k_norm_w, rope_cache, positions, cu_seqlens):
    global LAST_RESULT
    from concourse.bass_utils import run_bass_kernel_spmd  # noqa: PLC0415

    x = np.asarray(x, dtype=np.float32)
    wq = np.asarray(wq, dtype=np.float32)
    wk = np.asarray(wk, dtype=np.float32)
    wv = np.asarray(wv, dtype=np.float32)
    wo = np.asarray(wo, dtype=np.float32)
    q_norm_w = np.asarray(q_norm_w, dtype=np.float32)
    k_norm_w = np.asarray(k_norm_w, dtype=np.float32)
    rope_cache = np.asarray(rope_cache, dtype=np.float32)
    positions = np.asarray(positions)
    cu_seqlens = np.asarray(cu_seqlens)

    import ml_dtypes  # noqa: PLC0415

    # ---- host prep (shared) ----
    xT = np.ascontiguousarray(x[0].T.astype(ml_dtypes.bfloat16))  # [D, S]

    pos = positions.reshape(-1)
    cs = rope_cache[pos]               # [S, HALF, 2]
    cosT = cs[:, :, 0].T               # [HALF, S]
    sinT = cs[:, :, 1].T
    cs1 = np.concatenate([cosT, cosT], axis=0)    # [128, S]
    cs2 = np.concatenate([-sinT, sinT], axis=0)

    def fold(w):
        w = w.reshape(HD, 1)
        wsw = np.concatenate([w[HALF:], w[:HALF]], axis=0)
        return cs1 * w, cs2 * wsw

    cs1q, cs2q = fold(q_norm_w)
    cs1k, cs2k = fold(k_norm_w)
    cs_host = np.ascontiguousarray(
        np.concatenate([cs1q, cs2q, cs1k, cs2k], axis=1), dtype=np.float32
    )  # [128, 4S]

    plan, mask_pack = _attention_plan(cu_seqlens)

    consts = np.zeros((128, 5 * 128), dtype=np.float32)
    consts[:, 0:128] = 1.0
    swp = np.zeros((128, 128), dtype=np.float32)
    swp[np.arange(128), (np.arange(128) + HALF) % 128] = 1.0
    consts[:, 128:256] = swp
    consts[:, 256:384] = np.eye(128, dtype=np.float32)
    consts[:, 384:512] = np.triu(np.ones((128, 128), dtype=np.float32))
    consts[:, 512:640] = 1.0 / SCALE

    # ---- per-core weight shards ----
    in_maps = []
    for c in range(NCORES):
        w_all = np.concatenate(
            [
                wq[c * QH * HD:(c + 1) * QH * HD],   # [512, D]
                wk[c * HD:(c + 1) * HD],             # [128, D]
                wv[c * HD:(c + 1) * HD],             # [128, D]
            ],
            axis=0,
        )  # [NO*128, D]
        w_host = np.ascontiguousarray(
            w_all.reshape(NO, 128, DT, 128).transpose(3, 2, 0, 1)
            .reshape(128, NO * DT * 128).astype(ml_dtypes.bfloat16)
        )
        wo_c = wo[:, c * QH * HD:(c + 1) * QH * HD].T  # [512, D]
        wo_host = np.ascontiguousarray(
            wo_c.reshape(QH, 128, D).transpose(1, 0, 2)
            .reshape(128, QH * D).astype(ml_dtypes.bfloat16)
        )
        in_maps.append(
            {
                "xT": xT,
                "w_qkv": w_host,
                "w_o": wo_host,
                "cs": cs_host,
                "mask_pack": mask_pack,
                "consts": consts,
            }
        )

    nc = _build_graph(plan, mask_pack.shape[1])
    res = run_bass_kernel_spmd(nc, in_maps, list(range(NCORES)))
    LAST_RESULT = res

    out = res.results[0]["out"].astype(np.float32)
    for c in range(1, NCORES):
        out = out + res.results[c]["out"]
    return out.reshape(1, S, D)


# revision 14
# speedup vs baseline: 1.4761x; 1.0332x over previous
"""Trainium2 Bass kernel for nn_Attention_2216203124924 (sparse/varlen GQA attention).

Full computation:
  xq/xk/xv = x @ {wq,wk,wv}.T ; per-head RMSNorm(q,k) ; RoPE via
  rope_cache[positions] ; GQA repeat ; per-segment causal attention
  (segments from cu_seqlens) ; out @ wo.T

Distribution (8 NeuronCores, tensor-parallel over heads):
  core c owns q-heads [4c,4c+4) and kv-head c (GQA groups align),
  wo is row-sharded; each core emits a partial [2048,4096] output and the
  host sums the 8 partials.

On-device layout is "transposed" ([feature, seq]) throughout so the
contraction dim always sits on SBUF partitions:
  qT/kT/vT from weight-stationary projection matmuls, RMSNorm stats via
  ones-column matmul + matmul-broadcast of rsqrt row, RoPE as elementwise
  muls with host-gathered cos/sin (+ PE swap-half permutation), scores^T =
  kT_tile.T @ qT, probs via unnormalized exp (scores are O(1), max-sub
  unneeded) with compile-time segment mask plan, PV accumulated over key
  tiles in PSUM, normalization by matmul-broadcast reciprocal row, and the
  output projection from attnT tiles against wo^T.

All matmul operands are float32r (~13-bit mantissa, full PE rate).
The segment/causal structure from cu_seqlens and the rope gather by
positions are resolved on the host at build time; the NEFF is specialized
to them.
"""

import os
import sys

import numpy as np

for _p in ("/opt/trn_rl_repo",):
    if os.path.isdir(_p) and _p not in sys.path:
        sys.path.insert(0, _p)

S = 2048
D = 4096
HD = 128
HALF = 64
N_HEADS = 32
N_KV = 8
NCORES = 8
QH = N_HEADS // NCORES          # 4 q heads per core
NO = QH + 2                     # o-tiles per core in qkv projection: q0..q3, k, v
DT = D // 128                   # 32 contraction tiles
MC = S // 512                   # 4 m-chunks of 512
NT = S // 128                   # 16 key tiles
EPS = 1e-6
SCALE = HD ** -0.5

LAST_RESULT = None  # BassKernelResults of the most recent run (for test harness)


def _attention_plan(cu_seqlens):
    """Compile-time mask plan from cu_seqlens.

    Returns (plan, mask_pack):
      plan[mc] = list of (nt, w0, w1, mask_ops); w0/w1 are column offsets
      (multiples of 128, relative to the 512-wide m-chunk) of the contiguous
      valid window; mask_ops = [(j, kind, idx)] for 128-col subtiles needing
      a multiplicative 0/1 mask: kind 'tri' uses the shared causal triangle,
      kind 'host' uses mask_pack[:, idx*128:(idx+1)*128].
    """
    idx = np.arange(S)
    seg = np.searchsorted(np.asarray(cu_seqlens), idx, side="right") - 1
    mask_qk = (seg[:, None] == seg[None, :]) & (idx[:, None] >= idx[None, :])
    mask_t = mask_qk.T  # [n, m]

    plan = []
    tiles = []
    tile_ids = {}
    for mc in range(MC):
        entries = []
        for nt in range(NT):
            blk = mask_t[nt * 128:(nt + 1) * 128, mc * 512:(mc + 1) * 512]
            if not blk.any():
                continue
            js = [j for j in range(4) if blk[:, j * 128:(j + 1) * 128].any()]
            jlo, jhi = min(js), max(js)
            assert js == list(range(jlo, jhi + 1)), "valid window not contiguous"
            mops = []
            for j in range(jlo, jhi + 1):
                sub = blk[:, j * 128:(j + 1) * 128]
                if sub.all():
                    continue
                m0g = mc * 512 + j * 128
                n0g = nt * 128
                if m0g == n0g and np.array_equal(
                    sub, idx[:128][None, :] >= idx[:128][:, None]
                ):
                    mops.append((j, "tri", -1))
                else:
                    key = sub.tobytes()
                    if key not in tile_ids:
                        tile_ids[key] = len(tiles)
                        tiles.append(sub.astype(np.float32))
                    mops.append((j, "host", tile_ids[key]))
            entries.append((nt, jlo * 128, (jhi + 1) * 128, mops))
        assert entries, "every query row attends to at least itself"
        plan.append(entries)

    if tiles:
        mask_pack = np.concatenate(tiles, axis=1)
    else:
        mask_pack = np.zeros((128, 128), dtype=np.float32)
    return plan, np.ascontiguousarray(mask_pack)


def _build_graph(plan, n_mask_cols):
    import concourse.bass as bass  # noqa: PLC0415
    import concourse.mybir as mybir  # noqa: PLC0415
    import concourse.tile as tile  # noqa: PLC0415
    from concourse import bacc  # noqa: PLC0415
    from contextlib import ExitStack  # noqa: PLC0415

    f32 = mybir.dt.float32
    f32r = mybir.dt.float32r
    bf16 = mybir.dt.bfloat16
    AF = mybir.ActivationFunctionType

    nc = bacc.Bacc()
    xT_p = nc.declare_dram_parameter("xT", [D, S], bf16, isOutput=False)
    wqkv_p = nc.declare_dram_parameter("w_qkv", [128, NO * DT * 128], bf16, isOutput=False)
    wo_p = nc.declare_dram_parameter("w_o", [128, QH * D], bf16, isOutput=False)
    cs_p = nc.declare_dram_parameter("cs", [128, 4 * S], f32r, isOutput=False)
    mask_p = nc.declare_dram_parameter("mask_pack", [128, n_mask_cols], f32r, isOutput=False)
    consts_p = nc.declare_dram_parameter("consts", [128, 5 * 128], f32r, isOutput=False)
    out_p = nc.declare_dram_parameter("out", [S, D], f32, isOutput=True)

    with tile.TileContext(nc) as tc, ExitStack() as ctx:
        const = ctx.enter_context(tc.tile_pool(name="const", bufs=1))
        persist = ctx.enter_context(tc.tile_pool(name="persist", bufs=1))

        consts = const.tile([128, 5 * 128], f32r)
        nc.sync.dma_start(consts[:], consts_p[:])
        ones_col = consts[:, 0:1]
        ones_row = consts[0:1, 0:128]
        swp = consts[:, 128:256]        # swap-halves permutation
        ident = consts[:, 256:384]      # identity (for PE transpose)
        tri = consts[:, 384:512]        # causal triangle in [n, m]: 1 iff m >= n
        sca_row = consts[0:1, 512:640]  # all = HD**0.5 (divide-by folds the attn scale)

        mask_sb = const.tile([128, n_mask_cols], f32r)
        nc.sync.dma_start(mask_sb[:], mask_p[:])

        eps_col = const.tile([128, 1], f32)
        nc.gpsimd.memset(eps_col[:], EPS)

        # persistent activations: q0..q3, k, v in transposed [feat, seq] layout
        qkvT = [persist.tile([128, S], f32r, tag=f"qkvT{o}", name=f"qkvT{o}") for o in range(NO)]

        # ---------------- stage 1: qkv projection + rms stats ----------------
        with ExitStack() as s1:
            pw = s1.enter_context(tc.tile_pool(name="wqkv", bufs=1))
            px = s1.enter_context(tc.tile_pool(name="xstream", bufs=6))
            pq = s1.enter_context(tc.tile_pool(name="qkvpsum", bufs=1, space="PSUM"))

            w_sb = pw.tile([128, NO * DT * 128], bf16)
            wchunk = DT // 4 * NO * 128
            for wci in range(4):
                nc.sync.dma_start(
                    w_sb[:, wci * wchunk:(wci + 1) * wchunk],
                    wqkv_p[:, wci * wchunk:(wci + 1) * wchunk],
                )

            for mc in range(MC):
                msl = slice(mc * 512, (mc + 1) * 512)
                accs = [pq.tile([128, 512], f32, tag=f"acc{o}", name=f"acc{o}") for o in range(NO)]
                for d in range(DT):
                    xt = px.tile([128, 512], bf16, tag="xt")
                    nc.sync.dma_start(xt[:], xT_p[d * 128:(d + 1) * 128, msl])
                    for o in range(NO):
                        woff = (d * NO + o) * 128
                        nc.tensor.matmul(
                            accs[o][:],
                            w_sb[:, woff:woff + 128],
                            xt[:],
                            start=(d == 0),
                            stop=(d == DT - 1),
                        )
                for o in range(NO):
                    nc.vector.tensor_copy(qkvT[o][:, msl], accs[o][:])

        # ---------------- stage 2: rope + attention, interleaved per head ----------------
        with ExitStack() as s2:
            p2 = s2.enter_context(tc.tile_pool(name="persist2", bufs=1))
            v_sb = p2.tile([128, S], f32r)
            attnT = [p2.tile([128, S], bf16, tag=f"attnT{h}", name=f"attnT{h}") for h in range(QH)]
            wo_sb = p2.tile([128, QH * D], bf16)
            nc.sync.dma_start(wo_sb[:], wo_p[:])

            kT = qkvT[QH]
            vT = qkvT[QH + 1]

            with ExitStack() as s2b:
                pcs = s2b.enter_context(tc.tile_pool(name="csstream", bufs=2))
                psc = s2b.enter_context(tc.tile_pool(name="s2scratch", bufs=2))
                pss = s2b.enter_context(tc.tile_pool(name="ssqpsum", bufs=2, space="PSUM"))
                psco = s2b.enter_context(tc.tile_pool(name="scpsum", bufs=2, space="PSUM"))
                pov = s2b.enter_context(tc.tile_pool(name="ovpsum", bufs=1, space="PSUM"))
                pden = s2b.enter_context(tc.tile_pool(name="denpsum", bufs=1, space="PSUM"))
                pyp = s2b.enter_context(tc.tile_pool(name="ypsum", bufs=2, space="PSUM"))
                pex = s2b.enter_context(tc.tile_pool(name="exsbuf", bufs=3))
                pnr = s2b.enter_context(tc.tile_pool(name="nrsbuf", bufs=2))
                pys = s2b.enter_context(tc.tile_pool(name="ysbuf", bufs=3))

                def rope_chain(o, mcs=range(MC)):
                    csb = 0 if o < QH else 2
                    row = sca_row if o < QH else ones_row
                    for mc in mcs:
                        msl = slice(mc * 512, (mc + 1) * 512)
                        sq = psc.tile([128, 512], f32r, tag="sq", name="sq")
                        nc.scalar.activation(sq[:], qkvT[o][:, msl], AF.Square)
                        ss = pss.tile([1, 512], f32, tag="ssbc", name="ss", padded_shape=[128, 512])
                        nc.tensor.matmul(ss[:], ones_col, sq[:], start=True, stop=True)
                        rsq = psc.tile([1, 512], f32r, tag="rsq", name="rsq")
                        nc.scalar.activation(
                            rsq[:], ss[:], AF.Sqrt, bias=eps_col[0:1, :], scale=1.0 / HD
                        )
                        cs1 = pcs.tile([128, 512], f32r, tag="cs1", name="cs1")
                        cs2 = pcs.tile([128, 512], f32r, tag="cs2", name="cs2")
                        nc.sync.dma_start(cs1[:], cs_p[:, csb * S + mc * 512: csb * S + (mc + 1) * 512])
                        nc.sync.dma_start(cs2[:], cs_p[:, (csb + 1) * S + mc * 512: (csb + 1) * S + (mc + 1) * 512])
                        bp = pss.tile([128, 512], f32, tag="ssbc", name="bp")
                        nc.tensor.matmul(bp[:], swp, qkvT[o][:, msl], start=True, stop=True)
                        t1 = psc.tile([128, 512], f32, tag="t1", name="t1")
                        nc.vector.tensor_mul(t1[:], qkvT[o][:, msl], cs1[:])
                        t2 = psc.tile([128, 512], f32, tag="t2", name="t2")
                        nc.vector.tensor_mul(t2[:], bp[:], cs2[:])
                        nc.vector.tensor_add(t1[:], t1[:], t2[:])
                        bc = pss.tile([128, 512], f32, tag="ssbc", name="bc")
                        nc.tensor.matmul(bc[:], row, rsq[:], start=True, stop=True)
                        rrb = psc.tile([128, 512], f32, tag="rrb", name="rrb")
                        nc.vector.reciprocal_approx_fast(out=rrb[:], in_=bc[:])
                        nc.vector.tensor_mul(qkvT[o][:, msl], t1[:], rrb[:])

                # k first, then v transposes, then each q head followed by its attention
                rope_chain(QH)
                for nt in range(NT):
                    nsl = slice(nt * 128, (nt + 1) * 128)
                    tp = pss.tile([128, 128], f32, tag="ssbc", name="tp")
                    nc.tensor.transpose(
                        tp[:], vT[:, nsl].bitcast(mybir.dt.float32), ident.bitcast(mybir.dt.float32)
                    )
                    nc.vector.tensor_copy(v_sb[:, nsl], tp[:])

                for mc in range(MC):
                    for h in range(QH):
                        rope_chain(h, mcs=[mc])
                        entries = plan[mc]
                        ov = pov.tile([128, 512], f32, tag="ov")
                        den = pden.tile([1, 512], f32, tag="den")
                        n_ent = len(entries)
                        for i, (nt, w0, w1, mops) in enumerate(entries):
                            nsl = slice(nt * 128, (nt + 1) * 128)
                            qsl = slice(mc * 512 + w0, mc * 512 + w1)
                            sc = psco.tile([128, 512], f32, tag="sc")
                            nc.tensor.matmul(
                                sc[:, w0:w1], kT[:, nsl], qkvT[h][:, qsl],
                                start=True, stop=True,
                            )
                            ex = pex.tile([128, 512], f32r, tag="ex")
                            nc.scalar.activation(ex[:, w0:w1], sc[:, w0:w1], AF.Exp)
                            for (j, kind, tix) in mops:
                                jsl = slice(j * 128, (j + 1) * 128)
                                msrc = tri if kind == "tri" else mask_sb[:, tix * 128:(tix + 1) * 128]
                                nc.vector.tensor_mul(ex[:, jsl], ex[:, jsl], msrc)
                            first = i == 0
                            last = i == n_ent - 1
                            nc.tensor.matmul(
                                ov[:, w0:w1], v_sb[:, nsl], ex[:, w0:w1],
                                start=first, stop=last, skip_group_check=True,
                            )
                            nc.tensor.matmul(
                                den[0:1, w0:w1], ones_col, ex[:, w0:w1],
                                start=first, stop=last, skip_group_check=True,
                            )
                        den_sb = pnr.tile([1, 512], f32r, tag="den_sb")
                        nc.scalar.activation(den_sb[:], den[:], AF.Copy)
                        bc = pss.tile([128, 512], f32, tag="ssbc", name="bc2")
                        nc.tensor.matmul(bc[:], ones_row, den_sb[:], start=True, stop=True)
                        bcs = pnr.tile([128, 512], f32, tag="bcs")
                        nc.vector.reciprocal_approx_fast(out=bcs[:], in_=bc[:])
                        nc.vector.tensor_mul(
                            attnT[h][:, mc * 512:(mc + 1) * 512], ov[:], bcs[:]
                        )

                    # output projection for this mc (fills PE while next mc's rope runs)
                    for j in range(4):
                        mt = mc * 4 + j
                        tsl = slice(mt * 128, (mt + 1) * 128)
                        for ec in range(D // 512):
                            yp = pyp.tile([128, 512], f32, tag="yp", name="yp")
                            for t in range(QH):
                                nc.tensor.matmul(
                                    yp[:],
                                    attnT[t][:, tsl],
                                    wo_sb[:, t * D + ec * 512: t * D + (ec + 1) * 512],
                                    start=(t == 0),
                                    stop=(t == QH - 1),
                                )
                            ys = pys.tile([128, 512], f32, tag="ys", name="ys")
                            nc.scalar.activation(ys[:], yp[:], AF.Copy)
                            nc.sync.dma_start(out_p[tsl, ec * 512:(ec + 1) * 512], ys[:])

    nc.finalize()
    return nc


def kernel(x, wq, wk, wv, wo, q_norm_w, k_norm_w, rope_cache, positions, cu_seqlens):
    global LAST_RESULT
    from concourse.bass_utils import run_bass_kernel_spmd  # noqa: PLC0415

    x = np.asarray(x, dtype=np.float32)
    wq = np.asarray(wq, dtype=np.float32)
    wk = np.asarray(wk, dtype=np.float32)
    wv = np.asarray(wv, dtype=np.float32)
    wo = np.asarray(wo, dtype=np.float32)
    q_norm_w = np.asarray(q_norm_w, dtype=np.float32)
    k_norm_w = np.asarray(k_norm_w, dtype=np.float32)
    rope_cache = np.asarray(rope_cache, dtype=np.float32)
    positions = np.asarray(positions)
    cu_seqlens = np.asarray(cu_seqlens)

    import ml_dtypes  # noqa: PLC0415

    # ---- host prep (shared) ----
    xT = np.ascontiguousarray(x[0].T.astype(ml_dtypes.bfloat16))  # [D, S]

    pos = positions.reshape(-1)
    cs = rope_cache[pos]               # [S, HALF, 2]
    cosT = cs[:, :, 0].T               # [HALF, S]
    sinT = cs[:, :, 1].T
    cs1 = np.concatenate([cosT, cosT], axis=0)    # [128, S]
    cs2 = np.concatenate([-sinT, sinT], axis=0)

    def fold(w):
        w = w.reshape(HD, 1)
        wsw = np.concatenate([w[HALF:], w[:HALF]], axis=0)
        return cs1 * w, cs2 * wsw

    cs1q, cs2q = fold(q_norm_w)
    cs1k, cs2k = fold(k_norm_w)
    cs_host = np.ascontiguousarray(
        np.concatenate([cs1q, cs2q, cs1k, cs2k], axis=1), dtype=np.float32
    )  # [128, 4S]

    plan, mask_pack = _attention_plan(cu_seqlens)

    consts = np.zeros((128, 5 * 128), dtype=np.float32)
    consts[:, 0:128] = 1.0
    swp = np.zeros((128, 128), dtype=np.float32)
    swp[np.arange(128), (np.arange(128) + HALF) % 128] = 1.0
    consts[:, 128:256] = swp
    consts[:, 256:384] = np.eye(128, dtype=np.float32)
    consts[:, 384:512] = np.triu(np.ones((128, 128), dtype=np.float32))
    consts[:, 512:640] = 1.0 / SCALE

    # ---- per-core weight shards ----
    in_maps = []
    for c in range(NCORES):
        w_all = np.concatenate(
            [
                wq[c * QH * HD:(c + 1) * QH * HD],   # [512, D]
                wk[c * HD:(c + 1) * HD],             # [128, D]
                wv[c * HD:(c + 1) * HD],             # [128, D]
            ],
            axis=0,
        )  # [NO*128, D]
        w_host = np.ascontiguousarray(
            w_all.reshape(NO, 128, DT, 128).transpose(3, 2, 0, 1)
            .reshape(128, NO * DT * 128).astype(ml_dtypes.bfloat16)
        )
        wo_c = wo[:, c * QH * HD:(c + 1) * QH * HD].T  # [512, D]
        wo_host = np.ascontiguousarray(
            wo_c.reshape(QH, 128, D).transpose(1, 0, 2)
            .reshape(128, QH * D).astype(ml_dtypes.bfloat16)
        )
        in_maps.append(
            {
                "xT": xT,
                "w_qkv": w_host,
                "w_o": wo_host,
                "cs": cs_host,
                "mask_pack": mask_pack,
                "consts": consts,
            }
        )

    nc = _build_graph(plan, mask_pack.shape[1])
    res = run_bass_kernel_spmd(nc, in_maps, list(range(NCORES)))
    LAST_RESULT = res

    out = res.results[0]["out"].astype(np.float32)
    for c in range(1, NCORES):
        out = out + res.results[c]["out"]
    return out.reshape(1, S, D)
